# revision 21
# baseline (speedup 1.0000x reference)
"""Calibrated Spectral Mixer on 8 TRN2 NeuronCores (Bass/Tile).

Data-parallel over batch: 32 samples -> 4 per core.  Per sample:
  1. x (N,256) is PE-transposed into a zero-padded channel-major image
     xpT (256, 103*33) so the two 3x3 convs become 9 tap-shifted matmuls.
  2. fx conv and the FUSED (conv_x @ blockdiag(gate_w/temp)) "logits conv"
     are computed straight from xpT in (n, cout) orientation.
  3. softmax(logits) * inver -> eig (n-major), PE-transposed into eigT.
  4. spec = fx^T-contraction with eig via head-pair block matmuls,
     accumulated in SBUF; LayerNorm over (g,c); mlp; then the output
     projection is fused host-side-style on device:
     F[hg,co] = out_specT @ out_wT per head, out = eigT^T @ F + out_b.
"""

import numpy as np

H, W = 101, 31
HEADS, DH, FREQ = 8, 64, 64
C = 256
INNER = HEADS * DH          # 512
N = H * W                   # 3131
NCORES = 8
BPC = 4                     # samples per core
EPS = 1e-5
NLCR = 31 + N + 31          # one pad image-row at each end, flat layout

# n-tiles: 4 image rows (124 positions) each, last tile 1 row (31)
TILES = [(t, 124 * t, 4 * t, 4, 124) for t in range(25)] + [(25, 3100, 100, 1, 31)]
# (idx, n0, row0, nrows, cnt)

_BUILT = None
import os
WAVES = int(os.environ.get("KWAVES", "1"))
UNROLL_SAMPLES = os.environ.get("KUNROLL", "1") == "1"
STAGE = int(os.environ.get("KSTAGE", "7"))
KSUB = int(os.environ.get("KSUB", "9"))


def _build_program(bpc=BPC):
    import concourse.bacc as bacc
    import concourse.bass as bass
    import concourse.mybir as mybir
    from concourse.tile import TileContext
    from concourse.masks import make_identity

    dt = mybir.dt
    AF = mybir.ActivationFunctionType
    ALU = mybir.AluOpType
    ds = bass.ds

    nc = bacc.Bacc(None, target_bir_lowering=False)

    x_d = nc.declare_dram_parameter("x", (bpc * N, C), dt.float16, isOutput=False)
    wc_d = nc.declare_dram_parameter("wc", (2, 128, 9 * 1024), dt.float16, isOutput=False)
    cb_d = nc.declare_dram_parameter("cbias", (1, 1024), dt.float16, isOutput=False)
    inv_d = nc.declare_dram_parameter("inver", (N, FREQ), dt.float16, isOutput=False)
    mlp_d = nc.declare_dram_parameter("mlp", (128, DH), dt.float16, isOutput=False)
    gam_d = nc.declare_dram_parameter("gamT", (128, FREQ), dt.float32, isOutput=False)
    bet_d = nc.declare_dram_parameter("betT", (128, FREQ), dt.float32, isOutput=False)
    ow_d = nc.declare_dram_parameter("outwt", (DH, 8 * C), dt.float16, isOutput=False)
    ob_d = nc.declare_dram_parameter("outb", (1, C), dt.float16, isOutput=False)
    out_d = nc.declare_dram_parameter("out", (bpc * N, C), dt.float16, isOutput=True)

    with TileContext(nc) as tc:
        with (
            tc.tile_pool(name="consts", bufs=1) as consts,
            tc.tile_pool(name="pers", bufs=1) as pers,
            tc.tile_pool(name="xload", bufs=3) as xload,
            tc.tile_pool(name="fxsb", bufs=2) as fxsb,
            tc.tile_pool(name="expsb", bufs=2) as expsb,
            tc.tile_pool(name="eigsb", bufs=2) as eigsb,
            tc.tile_pool(name="smsb", bufs=2) as smsb,
            tc.tile_pool(name="outsb", bufs=3) as outsb,
            tc.tile_pool(name="lnsb", bufs=1) as lnsb,
            tc.tile_pool(name="psA", bufs=2, space="PSUM") as psA,
            tc.tile_pool(name="psB", bufs=2, space="PSUM") as psB,
            tc.tile_pool(name="psC", bufs=4, space="PSUM") as psC,
        ):
            # ---- constants ----
            wc_s = [consts.tile([128, 9 * 1024], dt.float16, tag=f"wc{k}", name=f"wc{k}") for k in range(2)]
            for k in range(2):
                nc.sync.dma_start(wc_s[k][:], wc_d[k])
            cb_s = consts.tile([1, 1024], dt.float16, tag="cb")
            nc.sync.dma_start(cb_s[:], cb_d[:])
            inv_h = consts.tile([124, 26 * FREQ], dt.float16, tag="invh")
            nc.sync.dma_start(
                inv_h[:, : 25 * FREQ].rearrange("p (t g) -> p t g", g=FREQ),
                inv_d[: 25 * 124].rearrange("(t p) g -> p t g", p=124),
            )
            nc.sync.dma_start(inv_h[:31, 25 * FREQ :], inv_d[25 * 124 :])
            inv_s = consts.tile([124, 26 * FREQ], dt.float32, tag="inv")
            nc.vector.tensor_copy(inv_s[:, :], inv_h[:, :])
            mlp_s = consts.tile([128, DH], dt.float16, tag="mlp")
            nc.sync.dma_start(mlp_s[:], mlp_d[:])
            gam_s = consts.tile([128, FREQ], dt.float32, tag="gam")
            nc.sync.dma_start(gam_s[:], gam_d[:])
            bet_s = consts.tile([128, FREQ], dt.float32, tag="bet")
            nc.sync.dma_start(bet_s[:], bet_d[:])
            ow_s = consts.tile([DH, 8 * C], dt.float16, tag="ow")
            nc.sync.dma_start(ow_s[:], ow_d[:])
            ob_s = consts.tile([1, C], dt.float16, tag="ob")
            nc.sync.dma_start(ob_s[:], ob_d[:])

            id_f = consts.tile([128, 128], dt.float32, tag="idf")
            make_identity(nc, id_f)
            id_b = consts.tile([128, 128], dt.float16, tag="idb")
            make_identity(nc, id_b)
            ones_b = consts.tile([1, 128], dt.float16, tag="onb")
            nc.gpsimd.memset(ones_b[:], 1.0)
            ones_cf = consts.tile([128, 1], dt.float32, tag="oncf")
            nc.gpsimd.memset(ones_cf[:], 1.0)
            ones_rf = consts.tile([1, 128], dt.float32, tag="onrf")
            nc.gpsimd.memset(ones_rf[:], 1.0)
            eps_t = consts.tile([128, 1], dt.float32, tag="eps")
            nc.gpsimd.memset(eps_t[:], EPS)
            ones_m = consts.tile([128, 128], dt.float32, tag="onm")
            nc.gpsimd.memset(ones_m[:], 1.0)

            # ---- persistent per-sample buffers ----
            # xq[k][d]: channel-major x, column-shifted by (d-1), one zero
            # image-row of padding at each end; tap (di,dj) of the conv is the
            # contiguous slice xq[k][dj][:, 31 + (row0+di-1)*31 : +cnt].
            xq = [[pers.tile([128, NLCR], dt.float16, tag=f"xq{k}{d}", name=f"xq{k}{d}")
                   for d in range(3)] for k in range(2)]
            for k in range(2):
                for d in range(3):
                    nc.gpsimd.memset(xq[k][d][:], 0.0)
            eigT = [pers.tile([128, N], dt.float16, tag=f"eigT{s}", name=f"eigT{s}") for s in range(4)]
            spec_acc = pers.tile([128, 4 * 128], dt.float32, tag="spacc")
            F_sb = [pers.tile([128, C], dt.float16, tag=f"F{p}", name=f"Fsb{p}") for p in range(4)]

            for iv in (list(range(bpc)) if UNROLL_SAMPLES else [None]):
              ctx_loop = tc.For_i(0, bpc, 1) if iv is None else None
              if ctx_loop is not None:
                iv = ctx_loop.__enter__()
              if True:
                # ---------- phase A: transpose x into channel-major + shifts ----------
                for (t, n0, row0, nrows, cnt) in (TILES if STAGE >= 2 else []):
                    xt = xload.tile([124, C], dt.float16, tag="xt")
                    nc.sync.dma_start(xt[:cnt, :], x_d[ds(iv * N + n0, cnt), :])
                    for k in range(2):
                        tp = psA.tile([128, 128], dt.float16, tag="a")
                        nc.tensor.transpose(
                            tp[:128, :cnt], xt[:cnt, k * 128 : (k + 1) * 128], id_b[:cnt, :cnt]
                        )
                        nc.scalar.copy(xq[k][1][:, 31 + n0 : 31 + n0 + cnt], tp[:, :cnt])
                for k in (range(2) if STAGE >= 2 else []):
                    c3 = xq[k][1][:, 31 : 31 + N].rearrange("c (i j) -> c i j", j=31)
                    l3 = xq[k][0][:, 31 : 31 + N].rearrange("c (i j) -> c i j", j=31)
                    r3 = xq[k][2][:, 31 : 31 + N].rearrange("c (i j) -> c i j", j=31)
                    nc.vector.tensor_copy(l3[:, :, 1:31], c3[:, :, 0:30])
                    nc.vector.tensor_copy(r3[:, :, 0:30], c3[:, :, 1:31])

                # ---------- phase B: conv + softmax + spec + eigT ----------
                for (t, n0, row0, nrows, cnt) in (TILES if STAGE >= 3 else []):
                    fxp = psA.tile([124, 512], dt.float32, tag="a")
                    lgp = psB.tile([124, 512], dt.float32, tag="b")
                    first = True
                    for k in range(2):
                        for tap in range(9):
                            di, dj = tap // 3, tap % 3
                            base = 31 + (row0 + di - 1) * 31
                            lhsT = xq[k][dj][:, base : base + cnt]
                            nc.tensor.matmul(
                                fxp[:cnt, :],
                                lhsT,
                                wc_s[k][:, tap * 1024 : tap * 1024 + 512],
                                start=first,
                                stop=False,
                            )
                            nc.tensor.matmul(
                                lgp[:cnt, :],
                                lhsT,
                                wc_s[k][:, tap * 1024 + 512 : tap * 1024 + 1024],
                                start=first,
                                stop=False,
                            )
                            first = False
                    nc.tensor.matmul(
                        fxp[:cnt, :], ones_b[:1, :cnt], cb_s[:1, :512], start=False, stop=True
                    )
                    nc.tensor.matmul(
                        lgp[:cnt, :], ones_b[:1, :cnt], cb_s[:1, 512:], start=False, stop=True
                    )
                    fx_t = fxsb.tile([124, 512], dt.float16, tag="fx")
                    nc.scalar.copy(fx_t[:cnt, :], fxp[:cnt, :])
                    if STAGE < 4:
                        nc.scalar.copy(fx_t[:cnt, :], lgp[:cnt, :])
                        continue

                    # softmax over each head's 64 freqs (no max-sub needed; logits are O(1))
                    ex = expsb.tile([124, 512], dt.float32, tag="ex")
                    sm = smsb.tile([124, 8], dt.float32, tag="sm")
                    for h in range(8):
                        nc.scalar.activation(
                            ex[:cnt, h * 64 : (h + 1) * 64],
                            lgp[:cnt, h * 64 : (h + 1) * 64],
                            AF.Exp,
                            accum_out=sm[:cnt, h : h + 1],
                        )
                    rs = smsb.tile([124, 8], dt.float32, tag="rs")
                    nc.vector.reciprocal(rs[:cnt, :], sm[:cnt, :])
                    eg = eigsb.tile([124, 512], dt.float16, tag="eg")
                    for h in range(8):
                        hs = slice(h * 64, (h + 1) * 64)
                        nc.vector.tensor_mul(
                            ex[:cnt, hs], ex[:cnt, hs],
                            inv_s[:cnt, t * 64 : (t + 1) * 64],
                        )
                        nc.vector.tensor_scalar(
                            eg[:cnt, hs], ex[:cnt, hs], rs[:cnt, h : h + 1], None, ALU.mult
                        )

                    # spec accumulation (head pairs, block matmul)
                    if STAGE < 5:
                        continue
                    for p in range(4):
                        ps = slice(p * 128, (p + 1) * 128)
                        sp = psC.tile([128, 128], dt.float32, tag="c")
                        nc.tensor.matmul(
                            sp[:, :], eg[:cnt, ps], fx_t[:cnt, ps], start=True, stop=True
                        )
                        if t == 0:
                            nc.vector.tensor_copy(spec_acc[:, ps], sp[:, :])
                        else:
                            nc.vector.tensor_add(spec_acc[:, ps], spec_acc[:, ps], sp[:, :])

                    # transpose eig into eigT
                    for s in range(4):
                        ss = slice(s * 128, (s + 1) * 128)
                        tp = psC.tile([128, 128], dt.float16, tag="c")
                        nc.tensor.transpose(tp[:128, :cnt], eg[:cnt, ss], id_b[:cnt, :cnt])
                        nc.scalar.copy(eigT[s][:, n0 : n0 + cnt], tp[:, :cnt])

                # ---------- LayerNorm over (g,c) per head + mlp + F ----------
                if STAGE < 6:
                    continue
                # specT pairs with off-diagonal quadrants zeroed so full-width
                # base-0 ones-matmuls give per-(h, g) column sums (and the
                # partition broadcast of the stats for free).
                stp = [lnsb.tile([128, 128], dt.float32, tag=f"stp{p}", name=f"stp{p}") for p in range(4)]
                sq = lnsb.tile([128, 128], dt.float32, tag="sq")
                s1v = lnsb.tile([128, 16], dt.float32, tag="s1v")  # [0:8]=S1 [8:16]=S2
                for p in range(4):
                    ps = slice(p * 128, (p + 1) * 128)
                    tp = psB.tile([128, 128], dt.float32, tag="b")
                    nc.tensor.transpose(tp[:, :], spec_acc[:, ps], id_f[:, :128])
                    nc.gpsimd.memset(stp[p][:, :], 0.0)
                    for q in range(2):
                        qp = slice(q * 64, (q + 1) * 64)
                        nc.scalar.copy(stp[p][qp, qp], tp[qp, qp])
                    nc.scalar.square(sq[:, :], stp[p][:, :])
                    if KSUB < 1:
                        continue
                    s1p = psB.tile([128, 128], dt.float32, tag="b")
                    s2p = psA.tile([128, 128], dt.float32, tag="a")
                    nc.tensor.matmul(s1p[:, :], ones_m[:, :], stp[p][:, :], start=True, stop=True)
                    nc.tensor.matmul(s2p[:, :], ones_m[:, :], sq[:, :], start=True, stop=True)
                    for q in range(2):
                        h = 2 * p + q
                        qp = slice(q * 64, (q + 1) * 64)
                        nc.vector.reduce_sum(
                            s1v[:, h : h + 1], s1p[:, qp], axis=mybir.AxisListType.X
                        )
                        nc.vector.reduce_sum(
                            s1v[:, 8 + h : 9 + h], s2p[:, qp], axis=mybir.AxisListType.X
                        )
                # stats replicated across all 128 partitions
                if KSUB < 2:
                    continue
                mu = lnsb.tile([128, 8], dt.float32, tag="mu")
                nc.vector.tensor_scalar(mu[:, :], s1v[:, :8], 1.0 / 4096.0, None, ALU.mult)
                ex2 = lnsb.tile([128, 8], dt.float32, tag="ex2")
                nc.vector.tensor_scalar(ex2[:, :], s1v[:, 8:], 1.0 / 4096.0, None, ALU.mult)
                musq = lnsb.tile([128, 8], dt.float32, tag="musq")
                nc.vector.tensor_mul(musq[:, :], mu[:, :], mu[:, :])
                var = lnsb.tile([128, 8], dt.float32, tag="var")
                nc.vector.tensor_sub(var[:, :], ex2[:, :], musq[:, :])
                stdv = lnsb.tile([128, 8], dt.float32, tag="stdv")
                nc.scalar.activation(stdv[:, :], var[:, :], AF.Sqrt, bias=eps_t[:, :1])
                rstd = lnsb.tile([128, 8], dt.float32, tag="rstd")
                nc.vector.reciprocal(rstd[:, :], stdv[:, :])

                if KSUB < 4:
                    continue
                stn8 = lnsb.tile([DH, 8 * DH], dt.float16, tag="stn8")
                ost8 = lnsb.tile([DH, 8 * DH], dt.float16, tag="ost8")
                for p in range(4):
                    stn = lnsb.tile([128, 128], dt.float16, tag=f"stn{p}", name=f"stn{p}")
                    for q in range(2):
                        h = 2 * p + q
                        qp = slice(q * 64, (q + 1) * 64)
                        nc.vector.tensor_scalar(
                            stp[p][qp, qp], stp[p][qp, qp],
                            mu[qp, h : h + 1], rstd[qp, h : h + 1],
                            ALU.subtract, ALU.mult,
                        )
                        nc.vector.tensor_mul(stp[p][qp, qp], stp[p][qp, qp], gam_s[qp, :])
                        nc.vector.tensor_add(stn[qp, qp], stp[p][qp, qp], bet_s[qp, :])
                    # gather normalized quadrants at base partition 0
                    nc.scalar.copy(stn8[:, (2 * p) * 64 : (2 * p + 1) * 64], stn[:64, :64])
                    nc.sync.dma_start(
                        stn8[:, (2 * p + 1) * 64 : (2 * p + 2) * 64], stn[64:128, 64:128]
                    )
                # mlp per head: out_specT[h] = mlp_w^T-contraction (all base 0)
                for h in (range(8) if KSUB >= 5 else []):
                    op_ = psB.tile([DH, DH], dt.float32, tag="b")
                    nc.tensor.matmul(
                        op_[:, :], mlp_s[:64, :], stn8[:, h * 64 : (h + 1) * 64],
                        start=True, stop=True,
                    )
                    nc.scalar.copy(ost8[:, h * 64 : (h + 1) * 64], op_[:, :])
                # F[hg, co] per head (all base 0; odd heads shifted via DMA)
                for h in (range(8) if KSUB >= 6 else []):
                    fp = psA.tile([64, C], dt.float32, tag="a")
                    nc.tensor.matmul(
                        fp[:, :], ost8[:, h * 64 : (h + 1) * 64],
                        ow_s[:, h * C : (h + 1) * C], start=True, stop=True,
                    )
                    if h % 2 == 0:
                        nc.scalar.copy(F_sb[h // 2][:64, :], fp[:, :])
                    else:
                        fstg = lnsb.tile([64, C], dt.float16, tag="fstg")
                        nc.scalar.copy(fstg[:, :], fp[:, :])
                        nc.sync.dma_start(F_sb[h // 2][64:128, :], fstg[:, :])

                # ---------- phase C: out = eigT^T @ F + out_b ----------
                for (t, n0, row0, nrows, cnt) in (TILES if STAGE >= 7 else []):
                    op_ = psA.tile([124, C], dt.float32, tag="a")
                    for s in range(4):
                        nc.tensor.matmul(
                            op_[:cnt, :], eigT[s][:, n0 : n0 + cnt], F_sb[s][:, :],
                            start=(s == 0), stop=False,
                        )
                    nc.tensor.matmul(
                        op_[:cnt, :], ones_b[:1, :cnt], ob_s[:1, :], start=False, stop=True
                    )
                    ot = outsb.tile([124, C], dt.float16, tag="ot")
                    nc.scalar.copy(ot[:cnt, :], op_[:cnt, :])
                    nc.sync.dma_start(out_d[ds(iv * N + n0, cnt), :], ot[:cnt, :])
              if ctx_loop is not None:
                ctx_loop.__exit__(None, None, None)

    nc.compile()
    return nc


def _host_prep(conv_fx_w, conv_fx_b, conv_x_w, conv_x_b, gate_w, gate_b,
               temperature, ln_gamma, ln_beta, mlp_w, out_w, out_b, inver):
    f16 = np.float16

    temp = np.clip(np.asarray(temperature, np.float32).reshape(HEADS), 0.1, 5.0)
    # Wbig[cout, h*64+g] = gate_w[g, cout%64... block-diag per head] / temp_h
    gw = np.asarray(gate_w, np.float32)          # (FREQ, DH) = (g, dh)
    wbig = np.zeros((INNER, INNER), np.float32)
    for h in range(HEADS):
        wbig[h * DH : (h + 1) * DH, h * FREQ : (h + 1) * FREQ] = gw.T / temp[h]
    # fused logits conv weights + bias
    wx = np.asarray(conv_x_w, np.float32)        # (cout, cin, 3, 3)
    wlog = np.einsum("oidj,oF->djiF", wx, wbig)  # (3,3,256,512)
    logb = np.asarray(conv_x_b, np.float32) @ wbig
    logb = logb + np.repeat(np.asarray(gate_b, np.float32)[None, :], HEADS, 0).reshape(-1) / np.repeat(temp, FREQ)
    wfx = np.asarray(conv_fx_w, np.float32).transpose(2, 3, 1, 0)  # (3,3,256,512)
    # combined (tap-major within k-half): (2, 128, 9, 1024)
    wc = np.concatenate([wfx, wlog], axis=-1)    # (3,3,256,1024)
    wc = wc.reshape(9, 2, 128, 1024).transpose(1, 2, 0, 3).reshape(2, 128, 9 * 1024)
    cbias = np.concatenate([np.asarray(conv_fx_b, np.float32), logb])[None, :]

    gamT = np.asarray(ln_gamma, np.float32).T    # (c, g)
    betT = np.asarray(ln_beta, np.float32).T
    mlp_rep = np.vstack([np.asarray(mlp_w, np.float32)] * 2)       # (128, 64)
    ow = np.asarray(out_w, np.float32)           # (256, 512)
    owt = ow.reshape(C, HEADS, DH).transpose(2, 1, 0).reshape(DH, HEADS * C)

    return {
        "wc": wc.astype(f16),
        "cbias": cbias.astype(f16),
        "inver": np.asarray(inver, np.float32).astype(f16),
        "mlp": mlp_rep.astype(f16),
        "gamT": np.ascontiguousarray(np.vstack([gamT, gamT])),
        "betT": np.ascontiguousarray(np.vstack([betT, betT])),
        "outwt": owt.astype(f16),
        "outb": np.asarray(out_b, np.float32)[None, :].astype(f16),
    }


def kernel(x, conv_fx_w, conv_fx_b, conv_x_w, conv_x_b, gate_w, gate_b,
           temperature, ln_gamma, ln_beta, mlp_w, out_w, out_b, inver):
    global _BUILT
    import time as _time
    import sys as _sys
    _t0 = _time.time()
    import concourse.bass2jax  # noqa: F401  (primes the exec path)
    _t1 = _time.time()
    if _BUILT is None:
        _BUILT = _build_program(BPC // WAVES)
    nc = _BUILT
    _t2 = _time.time()

    weights = _host_prep(conv_fx_w, conv_fx_b, conv_x_w, conv_x_b, gate_w, gate_b,
                         temperature, ln_gamma, ln_beta, mlp_w, out_w, out_b, inver)
    x = np.asarray(x).reshape(NCORES * BPC * N, C).astype(np.float16)

    _t3 = _time.time()
    out = _run_pjrt(nc, x, weights)
    _t4 = _time.time()
    out = out.reshape(NCORES * BPC, N, C).astype(np.float32)
    print(f"[kernel] imports={_t1-_t0:.2f}s build={_t2-_t1:.2f}s prep={_t3-_t2:.2f}s run={_t4-_t3:.2f}s gather={_time.time()-_t4:.2f}s",
          file=_sys.stderr, flush=True)
    return out

_JITTED = None


def _run_pjrt(nc, x_global, weights):
    """Sharded bass_exec run, split into KWAVES async waves so wave N's
    upload overlaps wave N-1's execute/download.  x/out sharded over cores,
    weights replicated, donated output buffers created on-device."""
    global _JITTED
    import jax
    import jax.numpy as jnp
    from jax.experimental.shard_map import shard_map
    from jax.sharding import Mesh, NamedSharding, PartitionSpec as P
    import concourse.mybir as mybir
    from concourse import bass2jax

    bass2jax.install_neuronx_cc_hook()

    bpc = BPC // WAVES
    pname = nc.partition_id_tensor.name if nc.partition_id_tensor else None
    in_names = []
    out_names = []
    out_shapes = []
    for alloc in nc.m.functions[0].allocations:
        if not isinstance(alloc, mybir.MemoryLocationSet):
            continue
        name = alloc.memorylocations[0].name
        if alloc.kind == "ExternalInput":
            if name != pname:
                in_names.append(name)
        elif alloc.kind == "ExternalOutput":
            out_shapes.append((tuple(alloc.tensor_shape), mybir.dt.np(alloc.dtype)))
            out_names.append(name)
    assert out_names == ["out"]
    import jax.core
    out_avals = [jax.core.ShapedArray(sh, dt) for sh, dt in out_shapes]
    all_in = list(in_names) + list(out_names)
    if pname is not None:
        all_in.append(pname)

    mesh = Mesh(np.asarray(jax.devices()[:NCORES]), ("core",))

    if _JITTED is None:
        def _body(*args):
            operands = list(args)
            if pname is not None:
                operands.append(bass2jax.partition_id_tensor())
            outs = bass2jax._bass_exec_p.bind(
                *operands,
                out_avals=tuple(out_avals),
                in_names=tuple(all_in),
                out_names=tuple(out_names),
                lowering_input_output_aliases=(),
                sim_require_finite=True,
                sim_require_nnan=True,
                nc=nc,
            )
            return tuple(outs)

        in_specs = tuple(
            P("core") if nm == "x" else P() for nm in in_names
        ) + (P("core"),)
        donate = (len(in_names),)
        sharded = jax.jit(
            shard_map(_body, mesh=mesh, in_specs=in_specs,
                      out_specs=(P("core"),), check_rep=False),
            donate_argnums=donate, keep_unused=True,
        )
        zfn = jax.jit(
            lambda: jnp.zeros((NCORES * bpc * N, C), jnp.float16),
            out_shardings=NamedSharding(mesh, P("core")),
        )
        _JITTED = (sharded, zfn)
    sharded, zfn = _JITTED

    wargs = [weights[nm] for nm in in_names if nm != "x"]
    xw = x_global.reshape(WAVES, NCORES * bpc * N, C)
    pend = []
    for w in range(WAVES):
        args = [xw[w] if nm == "x" else weights[nm] for nm in in_names]
        (oa,) = sharded(*args, zfn())
        pend.append(oa)
    outs = [np.asarray(oa) for oa in pend]
    return np.stack(outs, 0) if WAVES > 1 else outs[0]


# revision 22
# speedup vs baseline: 2.1483x; 2.1483x over previous
"""Calibrated Spectral Mixer on 8 TRN2 NeuronCores (Bass/Tile).

Data-parallel over batch: 32 samples -> 4 per core.  Per sample:
  1. x (N,256) is PE-transposed into a zero-padded channel-major image
     xpT (256, 103*33) so the two 3x3 convs become 9 tap-shifted matmuls.
  2. fx conv and the FUSED (conv_x @ blockdiag(gate_w/temp)) "logits conv"
     are computed straight from xpT in (n, cout) orientation.
  3. softmax(logits) * inver -> eig (n-major), PE-transposed into eigT.
  4. spec = fx^T-contraction with eig via head-pair block matmuls,
     accumulated in SBUF; LayerNorm over (g,c); mlp; then the output
     projection is fused host-side-style on device:
     F[hg,co] = out_specT @ out_wT per head, out = eigT^T @ F + out_b.
"""

import numpy as np

H, W = 101, 31
HEADS, DH, FREQ = 8, 64, 64
C = 256
INNER = HEADS * DH          # 512
N = H * W                   # 3131
NCORES = 8
BPC = 4                     # samples per core
EPS = 1e-5
NLCR = 31 + N + 31          # one pad image-row at each end, flat layout

# n-tiles: 4 image rows (124 positions) each, last tile 1 row (31)
TILES = [(t, 124 * t, 4 * t, 4, 124) for t in range(25)] + [(25, 3100, 100, 1, 31)]
# (idx, n0, row0, nrows, cnt)

_BUILT = None
import os
WAVES = int(os.environ.get("KWAVES", "1"))
UNROLL_SAMPLES = os.environ.get("KUNROLL", "1") == "1"
STAGE = int(os.environ.get("KSTAGE", "7"))
KSUB = int(os.environ.get("KSUB", "9"))


def _build_program(bpc=BPC):
    import concourse.bacc as bacc
    import concourse.bass as bass
    import concourse.mybir as mybir
    from concourse.tile import TileContext
    from concourse.masks import make_identity

    dt = mybir.dt
    AF = mybir.ActivationFunctionType
    ALU = mybir.AluOpType
    ds = bass.ds

    nc = bacc.Bacc(None, target_bir_lowering=False)

    x_d = nc.declare_dram_parameter("x", (bpc * N, C), dt.float16, isOutput=False)
    wc_d = nc.declare_dram_parameter("wc", (2, 128, 9 * 1024), dt.float16, isOutput=False)
    cb_d = nc.declare_dram_parameter("cbias", (1, 1024), dt.float16, isOutput=False)
    inv_d = nc.declare_dram_parameter("inver", (N, FREQ), dt.float16, isOutput=False)
    mlp_d = nc.declare_dram_parameter("mlp", (128, DH), dt.float16, isOutput=False)
    gam_d = nc.declare_dram_parameter("gamT", (128, FREQ), dt.float32, isOutput=False)
    bet_d = nc.declare_dram_parameter("betT", (128, FREQ), dt.float32, isOutput=False)
    ow_d = nc.declare_dram_parameter("outwt", (DH, 8 * C), dt.float16, isOutput=False)
    ob_d = nc.declare_dram_parameter("outb", (1, C), dt.float16, isOutput=False)
    out_d = nc.declare_dram_parameter("out", (bpc * N, C), dt.float16, isOutput=True)

    with TileContext(nc) as tc:
        with (
            tc.tile_pool(name="consts", bufs=1) as consts,
            tc.tile_pool(name="pers", bufs=1) as pers,
            tc.tile_pool(name="xload", bufs=3) as xload,
            tc.tile_pool(name="fxsb", bufs=2) as fxsb,
            tc.tile_pool(name="expsb", bufs=2) as expsb,
            tc.tile_pool(name="eigsb", bufs=2) as eigsb,
            tc.tile_pool(name="smsb", bufs=2) as smsb,
            tc.tile_pool(name="outsb", bufs=3) as outsb,
            tc.tile_pool(name="lnsb", bufs=1) as lnsb,
            tc.tile_pool(name="psA", bufs=2, space="PSUM") as psA,
            tc.tile_pool(name="psB", bufs=2, space="PSUM") as psB,
            tc.tile_pool(name="psC", bufs=4, space="PSUM") as psC,
        ):
            # ---- constants ----
            wc_s = [consts.tile([128, 9 * 1024], dt.float16, tag=f"wc{k}", name=f"wc{k}") for k in range(2)]
            for k in range(2):
                nc.sync.dma_start(wc_s[k][:], wc_d[k])
            cb_s = consts.tile([1, 1024], dt.float16, tag="cb")
            nc.sync.dma_start(cb_s[:], cb_d[:])
            inv_h = consts.tile([124, 26 * FREQ], dt.float16, tag="invh")
            nc.sync.dma_start(
                inv_h[:, : 25 * FREQ].rearrange("p (t g) -> p t g", g=FREQ),
                inv_d[: 25 * 124].rearrange("(t p) g -> p t g", p=124),
            )
            nc.sync.dma_start(inv_h[:31, 25 * FREQ :], inv_d[25 * 124 :])
            inv_s = consts.tile([124, 26 * FREQ], dt.float32, tag="inv")
            nc.vector.tensor_copy(inv_s[:, :], inv_h[:, :])
            mlp_s = consts.tile([128, DH], dt.float16, tag="mlp")
            nc.sync.dma_start(mlp_s[:], mlp_d[:])
            gam_s = consts.tile([128, FREQ], dt.float32, tag="gam")
            nc.sync.dma_start(gam_s[:], gam_d[:])
            bet_s = consts.tile([128, FREQ], dt.float32, tag="bet")
            nc.sync.dma_start(bet_s[:], bet_d[:])
            ow_s = consts.tile([DH, 8 * C], dt.float16, tag="ow")
            nc.sync.dma_start(ow_s[:], ow_d[:])
            ob_s = consts.tile([1, C], dt.float16, tag="ob")
            nc.sync.dma_start(ob_s[:], ob_d[:])

            id_f = consts.tile([128, 128], dt.float32, tag="idf")
            make_identity(nc, id_f)
            id_b = consts.tile([128, 128], dt.float16, tag="idb")
            make_identity(nc, id_b)
            ones_b = consts.tile([1, 128], dt.float16, tag="onb")
            nc.gpsimd.memset(ones_b[:], 1.0)
            ones_cf = consts.tile([128, 1], dt.float32, tag="oncf")
            nc.gpsimd.memset(ones_cf[:], 1.0)
            ones_rf = consts.tile([1, 128], dt.float32, tag="onrf")
            nc.gpsimd.memset(ones_rf[:], 1.0)
            eps_t = consts.tile([128, 1], dt.float32, tag="eps")
            nc.gpsimd.memset(eps_t[:], EPS)
            ones_m = consts.tile([128, 128], dt.float32, tag="onm")
            nc.gpsimd.memset(ones_m[:], 1.0)

            # ---- persistent per-sample buffers ----
            # xq[k][d]: channel-major x, column-shifted by (d-1), one zero
            # image-row of padding at each end; tap (di,dj) of the conv is the
            # contiguous slice xq[k][dj][:, 31 + (row0+di-1)*31 : +cnt].
            xq = [[pers.tile([128, NLCR], dt.float16, tag=f"xq{k}{d}", name=f"xq{k}{d}")
                   for d in range(3)] for k in range(2)]
            for k in range(2):
                for d in range(3):
                    nc.gpsimd.memset(xq[k][d][:], 0.0)
            eigT = [pers.tile([128, N], dt.float16, tag=f"eigT{s}", name=f"eigT{s}") for s in range(4)]
            spec_acc = pers.tile([128, 4 * 128], dt.float32, tag="spacc")
            F_sb = [pers.tile([128, C], dt.float16, tag=f"F{p}", name=f"Fsb{p}") for p in range(4)]

            for iv in (list(range(bpc)) if UNROLL_SAMPLES else [None]):
              ctx_loop = tc.For_i(0, bpc, 1) if iv is None else None
              if ctx_loop is not None:
                iv = ctx_loop.__enter__()
              if True:
                # ---------- phase A: transpose x into channel-major + shifts ----------
                for (t, n0, row0, nrows, cnt) in (TILES if STAGE >= 2 else []):
                    xt = xload.tile([124, C], dt.float16, tag="xt")
                    nc.sync.dma_start(xt[:cnt, :], x_d[ds(iv * N + n0, cnt), :])
                    for k in range(2):
                        tp = psA.tile([128, 128], dt.float16, tag="a")
                        nc.tensor.transpose(
                            tp[:128, :cnt], xt[:cnt, k * 128 : (k + 1) * 128], id_b[:cnt, :cnt]
                        )
                        nc.scalar.copy(xq[k][1][:, 31 + n0 : 31 + n0 + cnt], tp[:, :cnt])
                for k in (range(2) if STAGE >= 2 else []):
                    c3 = xq[k][1][:, 31 : 31 + N].rearrange("c (i j) -> c i j", j=31)
                    l3 = xq[k][0][:, 31 : 31 + N].rearrange("c (i j) -> c i j", j=31)
                    r3 = xq[k][2][:, 31 : 31 + N].rearrange("c (i j) -> c i j", j=31)
                    nc.vector.tensor_copy(l3[:, :, 1:31], c3[:, :, 0:30])
                    nc.vector.tensor_copy(r3[:, :, 0:30], c3[:, :, 1:31])

                # ---------- phase B: conv + softmax + spec + eigT ----------
                for (t, n0, row0, nrows, cnt) in (TILES if STAGE >= 3 else []):
                    fxp = psA.tile([124, 512], dt.float32, tag="a")
                    lgp = psB.tile([124, 512], dt.float32, tag="b")
                    first = True
                    for k in range(2):
                        for tap in range(9):
                            di, dj = tap // 3, tap % 3
                            base = 31 + (row0 + di - 1) * 31
                            lhsT = xq[k][dj][:, base : base + cnt]
                            nc.tensor.matmul(
                                fxp[:cnt, :],
                                lhsT,
                                wc_s[k][:, tap * 1024 : tap * 1024 + 512],
                                start=first,
                                stop=False,
                            )
                            nc.tensor.matmul(
                                lgp[:cnt, :],
                                lhsT,
                                wc_s[k][:, tap * 1024 + 512 : tap * 1024 + 1024],
                                start=first,
                                stop=False,
                            )
                            first = False
                    nc.tensor.matmul(
                        fxp[:cnt, :], ones_b[:1, :cnt], cb_s[:1, :512], start=False, stop=True
                    )
                    nc.tensor.matmul(
                        lgp[:cnt, :], ones_b[:1, :cnt], cb_s[:1, 512:], start=False, stop=True
                    )
                    fx_t = fxsb.tile([124, 512], dt.float16, tag="fx")
                    nc.scalar.copy(fx_t[:cnt, :], fxp[:cnt, :])
                    if STAGE < 4:
                        nc.scalar.copy(fx_t[:cnt, :], lgp[:cnt, :])
                        continue

                    # softmax over each head's 64 freqs (no max-sub needed; logits are O(1))
                    ex = expsb.tile([124, 512], dt.float32, tag="ex")
                    sm = smsb.tile([124, 8], dt.float32, tag="sm")
                    for h in range(8):
                        nc.scalar.activation(
                            ex[:cnt, h * 64 : (h + 1) * 64],
                            lgp[:cnt, h * 64 : (h + 1) * 64],
                            AF.Exp,
                            accum_out=sm[:cnt, h : h + 1],
                        )
                    rs = smsb.tile([124, 8], dt.float32, tag="rs")
                    nc.vector.reciprocal(rs[:cnt, :], sm[:cnt, :])
                    eg = eigsb.tile([124, 512], dt.float16, tag="eg")
                    for h in range(8):
                        hs = slice(h * 64, (h + 1) * 64)
                        nc.vector.tensor_mul(
                            ex[:cnt, hs], ex[:cnt, hs],
                            inv_s[:cnt, t * 64 : (t + 1) * 64],
                        )
                        nc.vector.tensor_scalar(
                            eg[:cnt, hs], ex[:cnt, hs], rs[:cnt, h : h + 1], None, ALU.mult
                        )

                    # spec accumulation (head pairs, block matmul)
                    if STAGE < 5:
                        continue
                    for p in range(4):
                        ps = slice(p * 128, (p + 1) * 128)
                        sp = psC.tile([128, 128], dt.float32, tag="c")
                        nc.tensor.matmul(
                            sp[:, :], eg[:cnt, ps], fx_t[:cnt, ps], start=True, stop=True
                        )
                        if t == 0:
                            nc.vector.tensor_copy(spec_acc[:, ps], sp[:, :])
                        else:
                            nc.vector.tensor_add(spec_acc[:, ps], spec_acc[:, ps], sp[:, :])

                    # transpose eig into eigT
                    for s in range(4):
                        ss = slice(s * 128, (s + 1) * 128)
                        tp = psC.tile([128, 128], dt.float16, tag="c")
                        nc.tensor.transpose(tp[:128, :cnt], eg[:cnt, ss], id_b[:cnt, :cnt])
                        nc.scalar.copy(eigT[s][:, n0 : n0 + cnt], tp[:, :cnt])

                # ---------- LayerNorm over (g,c) per head + mlp + F ----------
                if STAGE < 6:
                    continue
                # specT pairs with off-diagonal quadrants zeroed so full-width
                # base-0 ones-matmuls give per-(h, g) column sums (and the
                # partition broadcast of the stats for free).
                stp = [lnsb.tile([128, 128], dt.float32, tag=f"stp{p}", name=f"stp{p}") for p in range(4)]
                sq = lnsb.tile([128, 128], dt.float32, tag="sq")
                s1v = lnsb.tile([128, 16], dt.float32, tag="s1v")  # [0:8]=S1 [8:16]=S2
                for p in range(4):
                    ps = slice(p * 128, (p + 1) * 128)
                    tp = psB.tile([128, 128], dt.float32, tag="b")
                    nc.tensor.transpose(tp[:, :], spec_acc[:, ps], id_f[:, :128])
                    nc.gpsimd.memset(stp[p][:, :], 0.0)
                    for q in range(2):
                        qp = slice(q * 64, (q + 1) * 64)
                        nc.scalar.copy(stp[p][qp, qp], tp[qp, qp])
                    nc.scalar.square(sq[:, :], stp[p][:, :])
                    if KSUB < 1:
                        continue
                    s1p = psB.tile([128, 128], dt.float32, tag="b")
                    s2p = psA.tile([128, 128], dt.float32, tag="a")
                    nc.tensor.matmul(s1p[:, :], ones_m[:, :], stp[p][:, :], start=True, stop=True)
                    nc.tensor.matmul(s2p[:, :], ones_m[:, :], sq[:, :], start=True, stop=True)
                    for q in range(2):
                        h = 2 * p + q
                        qp = slice(q * 64, (q + 1) * 64)
                        nc.vector.reduce_sum(
                            s1v[:, h : h + 1], s1p[:, qp], axis=mybir.AxisListType.X
                        )
                        nc.vector.reduce_sum(
                            s1v[:, 8 + h : 9 + h], s2p[:, qp], axis=mybir.AxisListType.X
                        )
                # stats replicated across all 128 partitions
                if KSUB < 2:
                    continue
                mu = lnsb.tile([128, 8], dt.float32, tag="mu")
                nc.vector.tensor_scalar(mu[:, :], s1v[:, :8], 1.0 / 4096.0, None, ALU.mult)
                ex2 = lnsb.tile([128, 8], dt.float32, tag="ex2")
                nc.vector.tensor_scalar(ex2[:, :], s1v[:, 8:], 1.0 / 4096.0, None, ALU.mult)
                musq = lnsb.tile([128, 8], dt.float32, tag="musq")
                nc.vector.tensor_mul(musq[:, :], mu[:, :], mu[:, :])
                var = lnsb.tile([128, 8], dt.float32, tag="var")
                nc.vector.tensor_sub(var[:, :], ex2[:, :], musq[:, :])
                stdv = lnsb.tile([128, 8], dt.float32, tag="stdv")
                nc.scalar.activation(stdv[:, :], var[:, :], AF.Sqrt, bias=eps_t[:, :1])
                rstd = lnsb.tile([128, 8], dt.float32, tag="rstd")
                nc.vector.reciprocal(rstd[:, :], stdv[:, :])

                if KSUB < 4:
                    continue
                stn8 = lnsb.tile([DH, 8 * DH], dt.float16, tag="stn8")
                ost8 = lnsb.tile([DH, 8 * DH], dt.float16, tag="ost8")
                for p in range(4):
                    stn = lnsb.tile([128, 128], dt.float16, tag=f"stn{p}", name=f"stn{p}")
                    for q in range(2):
                        h = 2 * p + q
                        qp = slice(q * 64, (q + 1) * 64)
                        nc.vector.tensor_scalar(
                            stp[p][qp, qp], stp[p][qp, qp],
                            mu[qp, h : h + 1], rstd[qp, h : h + 1],
                            ALU.subtract, ALU.mult,
                        )
                        nc.vector.tensor_mul(stp[p][qp, qp], stp[p][qp, qp], gam_s[qp, :])
                        nc.vector.tensor_add(stn[qp, qp], stp[p][qp, qp], bet_s[qp, :])
                    # gather normalized quadrants at base partition 0
                    nc.scalar.copy(stn8[:, (2 * p) * 64 : (2 * p + 1) * 64], stn[:64, :64])
                    nc.sync.dma_start(
                        stn8[:, (2 * p + 1) * 64 : (2 * p + 2) * 64], stn[64:128, 64:128]
                    )
                # mlp per head: out_specT[h] = mlp_w^T-contraction (all base 0)
                for h in (range(8) if KSUB >= 5 else []):
                    op_ = psB.tile([DH, DH], dt.float32, tag="b")
                    nc.tensor.matmul(
                        op_[:, :], mlp_s[:64, :], stn8[:, h * 64 : (h + 1) * 64],
                        start=True, stop=True,
                    )
                    nc.scalar.copy(ost8[:, h * 64 : (h + 1) * 64], op_[:, :])
                # F[hg, co] per head (all base 0; odd heads shifted via DMA)
                for h in (range(8) if KSUB >= 6 else []):
                    fp = psA.tile([64, C], dt.float32, tag="a")
                    nc.tensor.matmul(
                        fp[:, :], ost8[:, h * 64 : (h + 1) * 64],
                        ow_s[:, h * C : (h + 1) * C], start=True, stop=True,
                    )
                    if h % 2 == 0:
                        nc.scalar.copy(F_sb[h // 2][:64, :], fp[:, :])
                    else:
                        fstg = lnsb.tile([64, C], dt.float16, tag="fstg")
                        nc.scalar.copy(fstg[:, :], fp[:, :])
                        nc.sync.dma_start(F_sb[h // 2][64:128, :], fstg[:, :])

                # ---------- phase C: out = eigT^T @ F + out_b ----------
                for (t, n0, row0, nrows, cnt) in (TILES if STAGE >= 7 else []):
                    op_ = psA.tile([124, C], dt.float32, tag="a")
                    for s in range(4):
                        nc.tensor.matmul(
                            op_[:cnt, :], eigT[s][:, n0 : n0 + cnt], F_sb[s][:, :],
                            start=(s == 0), stop=False,
                        )
                    nc.tensor.matmul(
                        op_[:cnt, :], ones_b[:1, :cnt], ob_s[:1, :], start=False, stop=True
                    )
                    ot = outsb.tile([124, C], dt.float16, tag="ot")
                    nc.scalar.copy(ot[:cnt, :], op_[:cnt, :])
                    nc.sync.dma_start(out_d[ds(iv * N + n0, cnt), :], ot[:cnt, :])
              if ctx_loop is not None:
                ctx_loop.__exit__(None, None, None)

    nc.compile()
    return nc


def _host_prep(conv_fx_w, conv_fx_b, conv_x_w, conv_x_b, gate_w, gate_b,
               temperature, ln_gamma, ln_beta, mlp_w, out_w, out_b, inver):
    f16 = np.float16

    temp = np.clip(np.asarray(temperature, np.float32).reshape(HEADS), 0.1, 5.0)
    # Wbig[cout, h*64+g] = gate_w[g, cout%64... block-diag per head] / temp_h
    gw = np.asarray(gate_w, np.float32)          # (FREQ, DH) = (g, dh)
    wbig = np.zeros((INNER, INNER), np.float32)
    for h in range(HEADS):
        wbig[h * DH : (h + 1) * DH, h * FREQ : (h + 1) * FREQ] = gw.T / temp[h]
    # fused logits conv weights + bias
    wx = np.asarray(conv_x_w, np.float32)        # (cout, cin, 3, 3)
    wlog = np.einsum("oidj,oF->djiF", wx, wbig)  # (3,3,256,512)
    logb = np.asarray(conv_x_b, np.float32) @ wbig
    logb = logb + np.repeat(np.asarray(gate_b, np.float32)[None, :], HEADS, 0).reshape(-1) / np.repeat(temp, FREQ)
    wfx = np.asarray(conv_fx_w, np.float32).transpose(2, 3, 1, 0)  # (3,3,256,512)
    # combined (tap-major within k-half): (2, 128, 9, 1024)
    wc = np.concatenate([wfx, wlog], axis=-1)    # (3,3,256,1024)
    wc = wc.reshape(9, 2, 128, 1024).transpose(1, 2, 0, 3).reshape(2, 128, 9 * 1024)
    cbias = np.concatenate([np.asarray(conv_fx_b, np.float32), logb])[None, :]

    gamT = np.asarray(ln_gamma, np.float32).T    # (c, g)
    betT = np.asarray(ln_beta, np.float32).T
    mlp_rep = np.vstack([np.asarray(mlp_w, np.float32)] * 2)       # (128, 64)
    ow = np.asarray(out_w, np.float32)           # (256, 512)
    owt = ow.reshape(C, HEADS, DH).transpose(2, 1, 0).reshape(DH, HEADS * C)

    return {
        "wc": wc.astype(f16),
        "cbias": cbias.astype(f16),
        "inver": np.asarray(inver, np.float32).astype(f16),
        "mlp": mlp_rep.astype(f16),
        "gamT": np.ascontiguousarray(np.vstack([gamT, gamT])),
        "betT": np.ascontiguousarray(np.vstack([betT, betT])),
        "outwt": owt.astype(f16),
        "outb": np.asarray(out_b, np.float32)[None, :].astype(f16),
    }


def kernel(x, conv_fx_w, conv_fx_b, conv_x_w, conv_x_b, gate_w, gate_b,
           temperature, ln_gamma, ln_beta, mlp_w, out_w, out_b, inver):
    global _BUILT
    import time as _time
    import sys as _sys
    _t0 = _time.time()
    import concourse.bass2jax  # noqa: F401  (primes the exec path)
    _t1 = _time.time()
    if _BUILT is None:
        _BUILT = _build_program(BPC // WAVES)
    nc = _BUILT
    _t2 = _time.time()

    weights = _host_prep(conv_fx_w, conv_fx_b, conv_x_w, conv_x_b, gate_w, gate_b,
                         temperature, ln_gamma, ln_beta, mlp_w, out_w, out_b, inver)
    x = np.asarray(x).reshape(NCORES * BPC * N, C).astype(np.float16)

    _t3 = _time.time()
    out = _run_pjrt(nc, x, weights)
    _t4 = _time.time()
    out = out.reshape(NCORES * BPC, N, C).astype(np.float32)
    print(f"[kernel] imports={_t1-_t0:.2f}s build={_t2-_t1:.2f}s prep={_t3-_t2:.2f}s run={_t4-_t3:.2f}s gather={_time.time()-_t4:.2f}s",
          file=_sys.stderr, flush=True)
    return out

_JITTED = None


def _run_pjrt(nc, x_global, weights):
    """Sharded bass_exec run, split into KWAVES async waves so wave N's
    upload overlaps wave N-1's execute/download.  x/out sharded over cores,
    weights replicated, donated output buffers created on-device."""
    global _JITTED
    import jax
    import jax.numpy as jnp
    from jax.experimental.shard_map import shard_map
    from jax.sharding import Mesh, NamedSharding, PartitionSpec as P
    import concourse.mybir as mybir
    from concourse import bass2jax

    bass2jax.install_neuronx_cc_hook()

    bpc = BPC // WAVES
    pname = nc.partition_id_tensor.name if nc.partition_id_tensor else None
    in_names = []
    out_names = []
    out_shapes = []
    for alloc in nc.m.functions[0].allocations:
        if not isinstance(alloc, mybir.MemoryLocationSet):
            continue
        name = alloc.memorylocations[0].name
        if alloc.kind == "ExternalInput":
            if name != pname:
                in_names.append(name)
        elif alloc.kind == "ExternalOutput":
            out_shapes.append((tuple(alloc.tensor_shape), mybir.dt.np(alloc.dtype)))
            out_names.append(name)
    assert out_names == ["out"]
    import jax.core
    out_avals = [jax.core.ShapedArray(sh, dt) for sh, dt in out_shapes]
    all_in = list(in_names) + list(out_names)
    if pname is not None:
        all_in.append(pname)

    mesh = Mesh(np.asarray(jax.devices()[:NCORES]), ("core",))

    if _JITTED is None:
        def _body(*args):
            operands = list(args)
            if pname is not None:
                operands.append(bass2jax.partition_id_tensor())
            outs = bass2jax._bass_exec_p.bind(
                *operands,
                out_avals=tuple(out_avals),
                in_names=tuple(all_in),
                out_names=tuple(out_names),
                lowering_input_output_aliases=(),
                sim_require_finite=True,
                sim_require_nnan=True,
                nc=nc,
            )
            return tuple(outs)

        in_specs = tuple(
            P("core") if nm == "x" else P() for nm in in_names
        ) + (P("core"),)
        donate = (len(in_names),)
        sharded = jax.jit(
            shard_map(_body, mesh=mesh, in_specs=in_specs,
                      out_specs=(P("core"),), check_rep=False),
            donate_argnums=donate, keep_unused=True,
        )
        zfn = jax.jit(
            lambda: jnp.zeros((NCORES * bpc * N, C), jnp.float16),
            out_shardings=NamedSharding(mesh, P("core")),
        )
        _JITTED = (sharded, zfn)
    sharded, zfn = _JITTED

    rep = NamedSharding(mesh, P())
    wdev = {nm: jax.device_put(weights[nm], rep) for nm in in_names if nm != "x"}
    xw = x_global.reshape(WAVES, NCORES * bpc * N, C)
    pend = []
    for w in range(WAVES):
        args = [xw[w] if nm == "x" else wdev[nm] for nm in in_names]
        (oa,) = sharded(*args, zfn())
        pend.append(oa)
    outs = [np.asarray(oa) for oa in pend]
    return np.stack(outs, 0) if WAVES > 1 else outs[0]


# revision 23
# speedup vs baseline: 2.1974x; 1.0229x over previous
"""Calibrated Spectral Mixer on 8 TRN2 NeuronCores (Bass/Tile).

Data-parallel over batch: 32 samples -> 4 per core.  Per sample:
  1. x (N,256) is PE-transposed into a zero-padded channel-major image
     xpT (256, 103*33) so the two 3x3 convs become 9 tap-shifted matmuls.
  2. fx conv and the FUSED (conv_x @ blockdiag(gate_w/temp)) "logits conv"
     are computed straight from xpT in (n, cout) orientation.
  3. softmax(logits) * inver -> eig (n-major), PE-transposed into eigT.
  4. spec = fx^T-contraction with eig via head-pair block matmuls,
     accumulated in SBUF; LayerNorm over (g,c); mlp; then the output
     projection is fused host-side-style on device:
     F[hg,co] = out_specT @ out_wT per head, out = eigT^T @ F + out_b.
"""

import numpy as np

H, W = 101, 31
HEADS, DH, FREQ = 8, 64, 64
C = 256
INNER = HEADS * DH          # 512
N = H * W                   # 3131
NCORES = 8
BPC = 4                     # samples per core
EPS = 1e-5
NLCR = 31 + N + 31          # one pad image-row at each end, flat layout

# n-tiles: 4 image rows (124 positions) each, last tile 1 row (31)
TILES = [(t, 124 * t, 4 * t, 4, 124) for t in range(25)] + [(25, 3100, 100, 1, 31)]
# (idx, n0, row0, nrows, cnt)

_BUILT = None
import os
WAVES = int(os.environ.get("KWAVES", "1"))
UNROLL_SAMPLES = os.environ.get("KUNROLL", "1") == "1"
STAGE = int(os.environ.get("KSTAGE", "7"))
KSUB = int(os.environ.get("KSUB", "9"))


def _build_program(bpc=BPC):
    import concourse.bacc as bacc
    import concourse.bass as bass
    import concourse.mybir as mybir
    from concourse.tile import TileContext
    from concourse.masks import make_identity

    dt = mybir.dt
    AF = mybir.ActivationFunctionType
    ALU = mybir.AluOpType
    ds = bass.ds

    nc = bacc.Bacc(None, target_bir_lowering=False)

    x_d = nc.declare_dram_parameter("x", (bpc * N, C), dt.float16, isOutput=False)
    wc_d = nc.declare_dram_parameter("wc", (2, 128, 9 * 1024), dt.float16, isOutput=False)
    cb_d = nc.declare_dram_parameter("cbias", (1, 1024), dt.float16, isOutput=False)
    inv_d = nc.declare_dram_parameter("inver", (N, FREQ), dt.float16, isOutput=False)
    mlp_d = nc.declare_dram_parameter("mlp", (128, DH), dt.float16, isOutput=False)
    gam_d = nc.declare_dram_parameter("gamT", (128, FREQ), dt.float32, isOutput=False)
    bet_d = nc.declare_dram_parameter("betT", (128, FREQ), dt.float32, isOutput=False)
    ow_d = nc.declare_dram_parameter("outwt", (DH, 8 * C), dt.float16, isOutput=False)
    ob_d = nc.declare_dram_parameter("outb", (1, C), dt.float16, isOutput=False)
    out_d = nc.declare_dram_parameter("out", (bpc * N, C), dt.float16, isOutput=True)

    with TileContext(nc) as tc:
        with (
            tc.tile_pool(name="consts", bufs=1) as consts,
            tc.tile_pool(name="pers", bufs=1) as pers,
            tc.tile_pool(name="xload", bufs=3) as xload,
            tc.tile_pool(name="fxsb", bufs=2) as fxsb,
            tc.tile_pool(name="expsb", bufs=2) as expsb,
            tc.tile_pool(name="eigsb", bufs=2) as eigsb,
            tc.tile_pool(name="smsb", bufs=2) as smsb,
            tc.tile_pool(name="outsb", bufs=3) as outsb,
            tc.tile_pool(name="lnsb", bufs=1) as lnsb,
            tc.tile_pool(name="psA", bufs=2, space="PSUM") as psA,
            tc.tile_pool(name="psB", bufs=2, space="PSUM") as psB,
            tc.tile_pool(name="psC", bufs=4, space="PSUM") as psC,
        ):
            # ---- constants ----
            wc_s = [consts.tile([128, 9 * 1024], dt.float16, tag=f"wc{k}", name=f"wc{k}") for k in range(2)]
            for k in range(2):
                nc.sync.dma_start(wc_s[k][:], wc_d[k])
            cb_s = consts.tile([1, 1024], dt.float16, tag="cb")
            nc.sync.dma_start(cb_s[:], cb_d[:])
            inv_h = consts.tile([124, 26 * FREQ], dt.float16, tag="invh")
            nc.sync.dma_start(
                inv_h[:, : 25 * FREQ].rearrange("p (t g) -> p t g", g=FREQ),
                inv_d[: 25 * 124].rearrange("(t p) g -> p t g", p=124),
            )
            nc.sync.dma_start(inv_h[:31, 25 * FREQ :], inv_d[25 * 124 :])
            inv_s = consts.tile([124, 26 * FREQ], dt.float32, tag="inv")
            nc.vector.tensor_copy(inv_s[:, :], inv_h[:, :])
            mlp_s = consts.tile([128, DH], dt.float16, tag="mlp")
            nc.sync.dma_start(mlp_s[:], mlp_d[:])
            gam_s = consts.tile([128, FREQ], dt.float32, tag="gam")
            nc.sync.dma_start(gam_s[:], gam_d[:])
            bet_s = consts.tile([128, FREQ], dt.float32, tag="bet")
            nc.sync.dma_start(bet_s[:], bet_d[:])
            ow_s = consts.tile([DH, 8 * C], dt.float16, tag="ow")
            nc.sync.dma_start(ow_s[:], ow_d[:])
            ob_s = consts.tile([1, C], dt.float16, tag="ob")
            nc.sync.dma_start(ob_s[:], ob_d[:])

            id_f = consts.tile([128, 128], dt.float32, tag="idf")
            make_identity(nc, id_f)
            id_b = consts.tile([128, 128], dt.float16, tag="idb")
            make_identity(nc, id_b)
            ones_b = consts.tile([1, 128], dt.float16, tag="onb")
            nc.gpsimd.memset(ones_b[:], 1.0)
            ones_cf = consts.tile([128, 1], dt.float32, tag="oncf")
            nc.gpsimd.memset(ones_cf[:], 1.0)
            ones_rf = consts.tile([1, 128], dt.float32, tag="onrf")
            nc.gpsimd.memset(ones_rf[:], 1.0)
            eps_t = consts.tile([128, 1], dt.float32, tag="eps")
            nc.gpsimd.memset(eps_t[:], EPS)
            ones_m = consts.tile([128, 128], dt.float32, tag="onm")
            nc.gpsimd.memset(ones_m[:], 1.0)

            # ---- persistent per-sample buffers ----
            # xq[k][d]: channel-major x, column-shifted by (d-1), one zero
            # image-row of padding at each end; tap (di,dj) of the conv is the
            # contiguous slice xq[k][dj][:, 31 + (row0+di-1)*31 : +cnt].
            xq = [[pers.tile([128, NLCR], dt.float16, tag=f"xq{k}{d}", name=f"xq{k}{d}")
                   for d in range(3)] for k in range(2)]
            for k in range(2):
                for d in range(3):
                    nc.gpsimd.memset(xq[k][d][:], 0.0)
            eigT = [pers.tile([128, N], dt.float16, tag=f"eigT{s}", name=f"eigT{s}") for s in range(4)]
            spec_acc = pers.tile([128, 4 * 128], dt.float32, tag="spacc")
            F_sb = [pers.tile([128, C], dt.float16, tag=f"F{p}", name=f"Fsb{p}") for p in range(4)]

            for iv in (list(range(bpc)) if UNROLL_SAMPLES else [None]):
              ctx_loop = tc.For_i(0, bpc, 1) if iv is None else None
              if ctx_loop is not None:
                iv = ctx_loop.__enter__()
              if True:
                # ---------- phase A: transpose x into channel-major + shifts ----------
                for (t, n0, row0, nrows, cnt) in (TILES if STAGE >= 2 else []):
                    xt = xload.tile([124, C], dt.float16, tag="xt")
                    nc.sync.dma_start(xt[:cnt, :], x_d[ds(iv * N + n0, cnt), :])
                    for k in range(2):
                        tp = psA.tile([128, 128], dt.float16, tag="a")
                        nc.tensor.transpose(
                            tp[:128, :cnt], xt[:cnt, k * 128 : (k + 1) * 128], id_b[:cnt, :cnt]
                        )
                        nc.scalar.copy(xq[k][1][:, 31 + n0 : 31 + n0 + cnt], tp[:, :cnt])
                for k in (range(2) if STAGE >= 2 else []):
                    c3 = xq[k][1][:, 31 : 31 + N].rearrange("c (i j) -> c i j", j=31)
                    l3 = xq[k][0][:, 31 : 31 + N].rearrange("c (i j) -> c i j", j=31)
                    r3 = xq[k][2][:, 31 : 31 + N].rearrange("c (i j) -> c i j", j=31)
                    nc.vector.tensor_copy(l3[:, :, 1:31], c3[:, :, 0:30])
                    nc.vector.tensor_copy(r3[:, :, 0:30], c3[:, :, 1:31])

                # ---------- phase B: conv + softmax + spec + eigT ----------
                for (t, n0, row0, nrows, cnt) in (TILES if STAGE >= 3 else []):
                    fxp = psA.tile([124, 512], dt.float32, tag="a")
                    lgp = psB.tile([124, 512], dt.float32, tag="b")
                    first = True
                    for k in range(2):
                        for tap in range(9):
                            di, dj = tap // 3, tap % 3
                            base = 31 + (row0 + di - 1) * 31
                            lhsT = xq[k][dj][:, base : base + cnt]
                            nc.tensor.matmul(
                                fxp[:cnt, :],
                                lhsT,
                                wc_s[k][:, tap * 1024 : tap * 1024 + 512],
                                start=first,
                                stop=False,
                            )
                            nc.tensor.matmul(
                                lgp[:cnt, :],
                                lhsT,
                                wc_s[k][:, tap * 1024 + 512 : tap * 1024 + 1024],
                                start=first,
                                stop=False,
                            )
                            first = False
                    nc.tensor.matmul(
                        fxp[:cnt, :], ones_b[:1, :cnt], cb_s[:1, :512], start=False, stop=True
                    )
                    nc.tensor.matmul(
                        lgp[:cnt, :], ones_b[:1, :cnt], cb_s[:1, 512:], start=False, stop=True
                    )
                    fx_t = fxsb.tile([124, 512], dt.float16, tag="fx")
                    nc.scalar.copy(fx_t[:cnt, :], fxp[:cnt, :])
                    if STAGE < 4:
                        nc.scalar.copy(fx_t[:cnt, :], lgp[:cnt, :])
                        continue

                    # softmax over each head's 64 freqs (no max-sub needed; logits are O(1))
                    ex = expsb.tile([124, 512], dt.float32, tag="ex")
                    sm = smsb.tile([124, 8], dt.float32, tag="sm")
                    for h in range(8):
                        nc.scalar.activation(
                            ex[:cnt, h * 64 : (h + 1) * 64],
                            lgp[:cnt, h * 64 : (h + 1) * 64],
                            AF.Exp,
                            accum_out=sm[:cnt, h : h + 1],
                        )
                    rs = smsb.tile([124, 8], dt.float32, tag="rs")
                    nc.vector.reciprocal(rs[:cnt, :], sm[:cnt, :])
                    eg = eigsb.tile([124, 512], dt.float16, tag="eg")
                    for h in range(8):
                        hs = slice(h * 64, (h + 1) * 64)
                        nc.vector.tensor_mul(
                            ex[:cnt, hs], ex[:cnt, hs],
                            inv_s[:cnt, t * 64 : (t + 1) * 64],
                        )
                        nc.vector.tensor_scalar(
                            eg[:cnt, hs], ex[:cnt, hs], rs[:cnt, h : h + 1], None, ALU.mult
                        )

                    # spec accumulation (head pairs, block matmul)
                    if STAGE < 5:
                        continue
                    for p in range(4):
                        ps = slice(p * 128, (p + 1) * 128)
                        sp = psC.tile([128, 128], dt.float32, tag="c")
                        nc.tensor.matmul(
                            sp[:, :], eg[:cnt, ps], fx_t[:cnt, ps], start=True, stop=True
                        )
                        if t == 0:
                            nc.vector.tensor_copy(spec_acc[:, ps], sp[:, :])
                        else:
                            nc.vector.tensor_add(spec_acc[:, ps], spec_acc[:, ps], sp[:, :])

                    # transpose eig into eigT
                    for s in range(4):
                        ss = slice(s * 128, (s + 1) * 128)
                        tp = psC.tile([128, 128], dt.float16, tag="c")
                        nc.tensor.transpose(tp[:128, :cnt], eg[:cnt, ss], id_b[:cnt, :cnt])
                        nc.scalar.copy(eigT[s][:, n0 : n0 + cnt], tp[:, :cnt])

                # ---------- LayerNorm over (g,c) per head + mlp + F ----------
                if STAGE < 6:
                    continue
                # specT pairs with off-diagonal quadrants zeroed so full-width
                # base-0 ones-matmuls give per-(h, g) column sums (and the
                # partition broadcast of the stats for free).
                stp = [lnsb.tile([128, 128], dt.float32, tag=f"stp{p}", name=f"stp{p}") for p in range(4)]
                sq = lnsb.tile([128, 128], dt.float32, tag="sq")
                s1v = lnsb.tile([128, 16], dt.float32, tag="s1v")  # [0:8]=S1 [8:16]=S2
                for p in range(4):
                    ps = slice(p * 128, (p + 1) * 128)
                    tp = psB.tile([128, 128], dt.float32, tag="b")
                    nc.tensor.transpose(tp[:, :], spec_acc[:, ps], id_f[:, :128])
                    nc.gpsimd.memset(stp[p][:, :], 0.0)
                    for q in range(2):
                        qp = slice(q * 64, (q + 1) * 64)
                        nc.scalar.copy(stp[p][qp, qp], tp[qp, qp])
                    nc.scalar.square(sq[:, :], stp[p][:, :])
                    if KSUB < 1:
                        continue
                    s1p = psB.tile([128, 128], dt.float32, tag="b")
                    s2p = psA.tile([128, 128], dt.float32, tag="a")
                    nc.tensor.matmul(s1p[:, :], ones_m[:, :], stp[p][:, :], start=True, stop=True)
                    nc.tensor.matmul(s2p[:, :], ones_m[:, :], sq[:, :], start=True, stop=True)
                    for q in range(2):
                        h = 2 * p + q
                        qp = slice(q * 64, (q + 1) * 64)
                        nc.vector.reduce_sum(
                            s1v[:, h : h + 1], s1p[:, qp], axis=mybir.AxisListType.X
                        )
                        nc.vector.reduce_sum(
                            s1v[:, 8 + h : 9 + h], s2p[:, qp], axis=mybir.AxisListType.X
                        )
                # stats replicated across all 128 partitions
                if KSUB < 2:
                    continue
                mu = lnsb.tile([128, 8], dt.float32, tag="mu")
                nc.vector.tensor_scalar(mu[:, :], s1v[:, :8], 1.0 / 4096.0, None, ALU.mult)
                ex2 = lnsb.tile([128, 8], dt.float32, tag="ex2")
                nc.vector.tensor_scalar(ex2[:, :], s1v[:, 8:], 1.0 / 4096.0, None, ALU.mult)
                musq = lnsb.tile([128, 8], dt.float32, tag="musq")
                nc.vector.tensor_mul(musq[:, :], mu[:, :], mu[:, :])
                var = lnsb.tile([128, 8], dt.float32, tag="var")
                nc.vector.tensor_sub(var[:, :], ex2[:, :], musq[:, :])
                stdv = lnsb.tile([128, 8], dt.float32, tag="stdv")
                nc.scalar.activation(stdv[:, :], var[:, :], AF.Sqrt, bias=eps_t[:, :1])
                rstd = lnsb.tile([128, 8], dt.float32, tag="rstd")
                nc.vector.reciprocal(rstd[:, :], stdv[:, :])

                if KSUB < 4:
                    continue
                stn8 = lnsb.tile([DH, 8 * DH], dt.float16, tag="stn8")
                ost8 = lnsb.tile([DH, 8 * DH], dt.float16, tag="ost8")
                for p in range(4):
                    stn = lnsb.tile([128, 128], dt.float16, tag=f"stn{p}", name=f"stn{p}")
                    for q in range(2):
                        h = 2 * p + q
                        qp = slice(q * 64, (q + 1) * 64)
                        nc.vector.tensor_scalar(
                            stp[p][qp, qp], stp[p][qp, qp],
                            mu[qp, h : h + 1], rstd[qp, h : h + 1],
                            ALU.subtract, ALU.mult,
                        )
                        nc.vector.tensor_mul(stp[p][qp, qp], stp[p][qp, qp], gam_s[qp, :])
                        nc.vector.tensor_add(stn[qp, qp], stp[p][qp, qp], bet_s[qp, :])
                    # gather normalized quadrants at base partition 0
                    nc.scalar.copy(stn8[:, (2 * p) * 64 : (2 * p + 1) * 64], stn[:64, :64])
                    nc.sync.dma_start(
                        stn8[:, (2 * p + 1) * 64 : (2 * p + 2) * 64], stn[64:128, 64:128]
                    )
                # mlp per head: out_specT[h] = mlp_w^T-contraction (all base 0)
                for h in (range(8) if KSUB >= 5 else []):
                    op_ = psB.tile([DH, DH], dt.float32, tag="b")
                    nc.tensor.matmul(
                        op_[:, :], mlp_s[:64, :], stn8[:, h * 64 : (h + 1) * 64],
                        start=True, stop=True,
                    )
                    nc.scalar.copy(ost8[:, h * 64 : (h + 1) * 64], op_[:, :])
                # F[hg, co] per head (all base 0; odd heads shifted via DMA)
                for h in (range(8) if KSUB >= 6 else []):
                    fp = psA.tile([64, C], dt.float32, tag="a")
                    nc.tensor.matmul(
                        fp[:, :], ost8[:, h * 64 : (h + 1) * 64],
                        ow_s[:, h * C : (h + 1) * C], start=True, stop=True,
                    )
                    if h % 2 == 0:
                        nc.scalar.copy(F_sb[h // 2][:64, :], fp[:, :])
                    else:
                        fstg = lnsb.tile([64, C], dt.float16, tag="fstg")
                        nc.scalar.copy(fstg[:, :], fp[:, :])
                        nc.sync.dma_start(F_sb[h // 2][64:128, :], fstg[:, :])

                # ---------- phase C: out = eigT^T @ F + out_b ----------
                for (t, n0, row0, nrows, cnt) in (TILES if STAGE >= 7 else []):
                    op_ = psA.tile([124, C], dt.float32, tag="a")
                    for s in range(4):
                        nc.tensor.matmul(
                            op_[:cnt, :], eigT[s][:, n0 : n0 + cnt], F_sb[s][:, :],
                            start=(s == 0), stop=False,
                        )
                    nc.tensor.matmul(
                        op_[:cnt, :], ones_b[:1, :cnt], ob_s[:1, :], start=False, stop=True
                    )
                    ot = outsb.tile([124, C], dt.float16, tag="ot")
                    nc.scalar.copy(ot[:cnt, :], op_[:cnt, :])
                    nc.sync.dma_start(out_d[ds(iv * N + n0, cnt), :], ot[:cnt, :])
              if ctx_loop is not None:
                ctx_loop.__exit__(None, None, None)

    nc.compile()
    return nc


def _host_prep(conv_fx_w, conv_fx_b, conv_x_w, conv_x_b, gate_w, gate_b,
               temperature, ln_gamma, ln_beta, mlp_w, out_w, out_b, inver):
    f16 = np.float16

    temp = np.clip(np.asarray(temperature, np.float32).reshape(HEADS), 0.1, 5.0)
    # Wbig[cout, h*64+g] = gate_w[g, cout%64... block-diag per head] / temp_h
    gw = np.asarray(gate_w, np.float32)          # (FREQ, DH) = (g, dh)
    wbig = np.zeros((INNER, INNER), np.float32)
    for h in range(HEADS):
        wbig[h * DH : (h + 1) * DH, h * FREQ : (h + 1) * FREQ] = gw.T / temp[h]
    # fused logits conv weights + bias
    wx = np.asarray(conv_x_w, np.float32)        # (cout, cin, 3, 3)
    wlog = np.einsum("oidj,oF->djiF", wx, wbig)  # (3,3,256,512)
    logb = np.asarray(conv_x_b, np.float32) @ wbig
    logb = logb + np.repeat(np.asarray(gate_b, np.float32)[None, :], HEADS, 0).reshape(-1) / np.repeat(temp, FREQ)
    wfx = np.asarray(conv_fx_w, np.float32).transpose(2, 3, 1, 0)  # (3,3,256,512)
    # combined (tap-major within k-half): (2, 128, 9, 1024)
    wc = np.concatenate([wfx, wlog], axis=-1)    # (3,3,256,1024)
    wc = wc.reshape(9, 2, 128, 1024).transpose(1, 2, 0, 3).reshape(2, 128, 9 * 1024)
    cbias = np.concatenate([np.asarray(conv_fx_b, np.float32), logb])[None, :]

    gamT = np.asarray(ln_gamma, np.float32).T    # (c, g)
    betT = np.asarray(ln_beta, np.float32).T
    mlp_rep = np.vstack([np.asarray(mlp_w, np.float32)] * 2)       # (128, 64)
    ow = np.asarray(out_w, np.float32)           # (256, 512)
    owt = ow.reshape(C, HEADS, DH).transpose(2, 1, 0).reshape(DH, HEADS * C)

    return {
        "wc": wc.astype(f16),
        "cbias": cbias.astype(f16),
        "inver": np.asarray(inver, np.float32).astype(f16),
        "mlp": mlp_rep.astype(f16),
        "gamT": np.ascontiguousarray(np.vstack([gamT, gamT])),
        "betT": np.ascontiguousarray(np.vstack([betT, betT])),
        "outwt": owt.astype(f16),
        "outb": np.asarray(out_b, np.float32)[None, :].astype(f16),
    }


def kernel(x, conv_fx_w, conv_fx_b, conv_x_w, conv_x_b, gate_w, gate_b,
           temperature, ln_gamma, ln_beta, mlp_w, out_w, out_b, inver):
    global _BUILT
    import time as _time
    import sys as _sys
    _t0 = _time.time()
    import concourse.bass2jax  # noqa: F401  (primes the exec path)
    _t1 = _time.time()
    if _BUILT is None:
        _BUILT = _build_program(BPC // WAVES)
    nc = _BUILT
    _t2 = _time.time()

    weights = _host_prep(conv_fx_w, conv_fx_b, conv_x_w, conv_x_b, gate_w, gate_b,
                         temperature, ln_gamma, ln_beta, mlp_w, out_w, out_b, inver)
    x = np.asarray(x).reshape(NCORES * BPC * N, C).astype(np.float16)

    _t3 = _time.time()
    out = _run_pjrt(nc, x, weights)
    _t4 = _time.time()
    out = out.reshape(NCORES * BPC, N, C).astype(np.float32)
    print(f"[kernel] imports={_t1-_t0:.2f}s build={_t2-_t1:.2f}s prep={_t3-_t2:.2f}s run={_t4-_t3:.2f}s gather={_time.time()-_t4:.2f}s",
          file=_sys.stderr, flush=True)
    return out

_JITTED = None


def _run_pjrt(nc, x_global, weights):
    """Sharded bass_exec run, split into KWAVES async waves so wave N's
    upload overlaps wave N-1's execute/download.  x/out sharded over cores,
    weights replicated, donated output buffers created on-device."""
    global _JITTED
    import jax
    import jax.numpy as jnp
    from jax.experimental.shard_map import shard_map
    from jax.sharding import Mesh, NamedSharding, PartitionSpec as P
    import concourse.mybir as mybir
    from concourse import bass2jax

    bass2jax.install_neuronx_cc_hook()
    try:
        jax.config.update("jax_compilation_cache_dir", "/tmp/jax_comp_cache")
        jax.config.update("jax_persistent_cache_min_compile_time_secs", 0.0)
        jax.config.update("jax_persistent_cache_min_entry_size_bytes", -1)
    except Exception:
        pass

    bpc = BPC // WAVES
    pname = nc.partition_id_tensor.name if nc.partition_id_tensor else None
    in_names = []
    out_names = []
    out_shapes = []
    for alloc in nc.m.functions[0].allocations:
        if not isinstance(alloc, mybir.MemoryLocationSet):
            continue
        name = alloc.memorylocations[0].name
        if alloc.kind == "ExternalInput":
            if name != pname:
                in_names.append(name)
        elif alloc.kind == "ExternalOutput":
            out_shapes.append((tuple(alloc.tensor_shape), mybir.dt.np(alloc.dtype)))
            out_names.append(name)
    assert out_names == ["out"]
    import jax.core
    out_avals = [jax.core.ShapedArray(sh, dt) for sh, dt in out_shapes]
    all_in = list(in_names) + list(out_names)
    if pname is not None:
        all_in.append(pname)

    mesh = Mesh(np.asarray(jax.devices()[:NCORES]), ("core",))

    if _JITTED is None:
        def _body(*args):
            operands = list(args)
            if pname is not None:
                operands.append(bass2jax.partition_id_tensor())
            outs = bass2jax._bass_exec_p.bind(
                *operands,
                out_avals=tuple(out_avals),
                in_names=tuple(all_in),
                out_names=tuple(out_names),
                lowering_input_output_aliases=(),
                sim_require_finite=True,
                sim_require_nnan=True,
                nc=nc,
            )
            return tuple(outs)

        in_specs = tuple(
            P("core") if nm == "x" else P() for nm in in_names
        ) + (P("core"),)
        donate = (len(in_names),)
        sharded = jax.jit(
            shard_map(_body, mesh=mesh, in_specs=in_specs,
                      out_specs=(P("core"),), check_rep=False),
            donate_argnums=donate, keep_unused=True,
        )
        zfn = jax.jit(
            lambda: jnp.zeros((NCORES * bpc * N, C), jnp.float16),
            out_shardings=NamedSharding(mesh, P("core")),
        )
        _JITTED = (sharded, zfn)
    sharded, zfn = _JITTED

    rep = NamedSharding(mesh, P())
    wdev = {nm: jax.device_put(weights[nm], rep) for nm in in_names if nm != "x"}
    xw = x_global.reshape(WAVES, NCORES * bpc * N, C)
    pend = []
    for w in range(WAVES):
        args = [xw[w] if nm == "x" else wdev[nm] for nm in in_names]
        (oa,) = sharded(*args, zfn())
        pend.append(oa)
    outs = [np.asarray(oa) for oa in pend]
    return np.stack(outs, 0) if WAVES > 1 else outs[0]


# revision 24
# speedup vs baseline: 2.6477x; 1.2049x over previous
"""Calibrated Spectral Mixer on 8 TRN2 NeuronCores (Bass/Tile).

Data-parallel over batch: 32 samples -> 4 per core.  Per sample:
  1. x (N,256) is PE-transposed into a zero-padded channel-major image
     xpT (256, 103*33) so the two 3x3 convs become 9 tap-shifted matmuls.
  2. fx conv and the FUSED (conv_x @ blockdiag(gate_w/temp)) "logits conv"
     are computed straight from xpT in (n, cout) orientation.
  3. softmax(logits) * inver -> eig (n-major), PE-transposed into eigT.
  4. spec = fx^T-contraction with eig via head-pair block matmuls,
     accumulated in SBUF; LayerNorm over (g,c); mlp; then the output
     projection is fused host-side-style on device:
     F[hg,co] = out_specT @ out_wT per head, out = eigT^T @ F + out_b.
"""

import numpy as np

H, W = 101, 31
HEADS, DH, FREQ = 8, 64, 64
C = 256
INNER = HEADS * DH          # 512
N = H * W                   # 3131
NCORES = 8
BPC = 4                     # samples per core
EPS = 1e-5
NLCR = 31 + N + 31          # one pad image-row at each end, flat layout

# n-tiles: 4 image rows (124 positions) each, last tile 1 row (31)
TILES = [(t, 124 * t, 4 * t, 4, 124) for t in range(25)] + [(25, 3100, 100, 1, 31)]
# (idx, n0, row0, nrows, cnt)

OFF_WC = 0
OFF_CB = OFF_WC + 2 * 128 * 9216
OFF_INV = OFF_CB + 1024
OFF_MLP = OFF_INV + N * FREQ
OFF_GAM = OFF_MLP + 128 * DH
OFF_BET = OFF_GAM + 128 * FREQ
OFF_OW = OFF_BET + 128 * FREQ
OFF_OB = OFF_OW + DH * 8 * C
WPACK_LEN = OFF_OB + C
assert WPACK_LEN % 8 == 0

_BUILT = None
import os
WAVES = int(os.environ.get("KWAVES", "1"))
UNROLL_SAMPLES = os.environ.get("KUNROLL", "1") == "1"
STAGE = int(os.environ.get("KSTAGE", "7"))
KSUB = int(os.environ.get("KSUB", "9"))


def _build_program(bpc=BPC):
    import concourse.bacc as bacc
    import concourse.bass as bass
    import concourse.mybir as mybir
    from concourse.tile import TileContext
    from concourse.masks import make_identity

    dt = mybir.dt
    AF = mybir.ActivationFunctionType
    ALU = mybir.AluOpType
    ds = bass.ds

    nc = bacc.Bacc(None, target_bir_lowering=False)

    x_d = nc.declare_dram_parameter("x", (bpc * N, C), dt.float16, isOutput=False)
    wp_d = nc.declare_dram_parameter("wpack", (WPACK_LEN,), dt.float16, isOutput=False)
    out_d = nc.declare_dram_parameter("out", (bpc * N, C), dt.float16, isOutput=True)

    def wslice(off, ln):
        return wp_d[off : off + ln]

    with TileContext(nc) as tc:
        with (
            tc.tile_pool(name="consts", bufs=1) as consts,
            tc.tile_pool(name="pers", bufs=1) as pers,
            tc.tile_pool(name="xload", bufs=3) as xload,
            tc.tile_pool(name="fxsb", bufs=2) as fxsb,
            tc.tile_pool(name="expsb", bufs=2) as expsb,
            tc.tile_pool(name="eigsb", bufs=2) as eigsb,
            tc.tile_pool(name="smsb", bufs=2) as smsb,
            tc.tile_pool(name="outsb", bufs=3) as outsb,
            tc.tile_pool(name="lnsb", bufs=1) as lnsb,
            tc.tile_pool(name="psA", bufs=2, space="PSUM") as psA,
            tc.tile_pool(name="psB", bufs=2, space="PSUM") as psB,
            tc.tile_pool(name="psC", bufs=4, space="PSUM") as psC,
        ):
            # ---- constants ----
            wc_s = [consts.tile([128, 9 * 1024], dt.float16, tag=f"wc{k}", name=f"wc{k}") for k in range(2)]
            for k in range(2):
                nc.sync.dma_start(
                    wc_s[k][:],
                    wslice(OFF_WC + k * 128 * 9216, 128 * 9216).rearrange("(p f) -> p f", f=9216),
                )
            cb_s = consts.tile([1, 1024], dt.float16, tag="cb")
            nc.sync.dma_start(cb_s[:], wslice(OFF_CB, 1024).rearrange("(p f) -> p f", p=1))
            inv_h = consts.tile([124, 26 * FREQ], dt.float16, tag="invh")
            nc.sync.dma_start(
                inv_h[:, : 25 * FREQ].rearrange("p (t g) -> p t g", g=FREQ),
                wslice(OFF_INV, 25 * 124 * FREQ).rearrange("(t p g) -> p t g", p=124, g=FREQ),
            )
            nc.sync.dma_start(
                inv_h[:31, 25 * FREQ :],
                wslice(OFF_INV + 25 * 124 * FREQ, 31 * FREQ).rearrange("(p g) -> p g", g=FREQ),
            )
            inv_s = consts.tile([124, 26 * FREQ], dt.float32, tag="inv")
            nc.vector.tensor_copy(inv_s[:, :], inv_h[:, :])
            mlp_s = consts.tile([128, DH], dt.float16, tag="mlp")
            nc.sync.dma_start(mlp_s[:], wslice(OFF_MLP, 128 * DH).rearrange("(p f) -> p f", f=DH))
            gam_h = consts.tile([128, FREQ], dt.float16, tag="gamh")
            nc.sync.dma_start(gam_h[:], wslice(OFF_GAM, 128 * FREQ).rearrange("(p f) -> p f", f=FREQ))
            gam_s = consts.tile([128, FREQ], dt.float32, tag="gam")
            nc.vector.tensor_copy(gam_s[:, :], gam_h[:, :])
            bet_h = consts.tile([128, FREQ], dt.float16, tag="beth")
            nc.sync.dma_start(bet_h[:], wslice(OFF_BET, 128 * FREQ).rearrange("(p f) -> p f", f=FREQ))
            bet_s = consts.tile([128, FREQ], dt.float32, tag="bet")
            nc.vector.tensor_copy(bet_s[:, :], bet_h[:, :])
            ow_s = consts.tile([DH, 8 * C], dt.float16, tag="ow")
            nc.sync.dma_start(ow_s[:], wslice(OFF_OW, DH * 8 * C).rearrange("(p f) -> p f", f=8 * C))
            ob_s = consts.tile([1, C], dt.float16, tag="ob")
            nc.sync.dma_start(ob_s[:], wslice(OFF_OB, C).rearrange("(p f) -> p f", p=1))

            id_f = consts.tile([128, 128], dt.float32, tag="idf")
            make_identity(nc, id_f)
            id_b = consts.tile([128, 128], dt.float16, tag="idb")
            make_identity(nc, id_b)
            ones_b = consts.tile([1, 128], dt.float16, tag="onb")
            nc.gpsimd.memset(ones_b[:], 1.0)
            ones_cf = consts.tile([128, 1], dt.float32, tag="oncf")
            nc.gpsimd.memset(ones_cf[:], 1.0)
            ones_rf = consts.tile([1, 128], dt.float32, tag="onrf")
            nc.gpsimd.memset(ones_rf[:], 1.0)
            eps_t = consts.tile([128, 1], dt.float32, tag="eps")
            nc.gpsimd.memset(eps_t[:], EPS)
            ones_m = consts.tile([128, 128], dt.float32, tag="onm")
            nc.gpsimd.memset(ones_m[:], 1.0)

            # ---- persistent per-sample buffers ----
            # xq[k][d]: channel-major x, column-shifted by (d-1), one zero
            # image-row of padding at each end; tap (di,dj) of the conv is the
            # contiguous slice xq[k][dj][:, 31 + (row0+di-1)*31 : +cnt].
            xq = [[pers.tile([128, NLCR], dt.float16, tag=f"xq{k}{d}", name=f"xq{k}{d}")
                   for d in range(3)] for k in range(2)]
            for k in range(2):
                for d in range(3):
                    nc.gpsimd.memset(xq[k][d][:], 0.0)
            eigT = [pers.tile([128, N], dt.float16, tag=f"eigT{s}", name=f"eigT{s}") for s in range(4)]
            spec_acc = pers.tile([128, 4 * 128], dt.float32, tag="spacc")
            F_sb = [pers.tile([128, C], dt.float16, tag=f"F{p}", name=f"Fsb{p}") for p in range(4)]

            for iv in (list(range(bpc)) if UNROLL_SAMPLES else [None]):
              ctx_loop = tc.For_i(0, bpc, 1) if iv is None else None
              if ctx_loop is not None:
                iv = ctx_loop.__enter__()
              if True:
                # ---------- phase A: transpose x into channel-major + shifts ----------
                for (t, n0, row0, nrows, cnt) in (TILES if STAGE >= 2 else []):
                    xt = xload.tile([124, C], dt.float16, tag="xt")
                    nc.sync.dma_start(xt[:cnt, :], x_d[ds(iv * N + n0, cnt), :])
                    for k in range(2):
                        tp = psA.tile([128, 128], dt.float16, tag="a")
                        nc.tensor.transpose(
                            tp[:128, :cnt], xt[:cnt, k * 128 : (k + 1) * 128], id_b[:cnt, :cnt]
                        )
                        nc.scalar.copy(xq[k][1][:, 31 + n0 : 31 + n0 + cnt], tp[:, :cnt])
                for k in (range(2) if STAGE >= 2 else []):
                    c3 = xq[k][1][:, 31 : 31 + N].rearrange("c (i j) -> c i j", j=31)
                    l3 = xq[k][0][:, 31 : 31 + N].rearrange("c (i j) -> c i j", j=31)
                    r3 = xq[k][2][:, 31 : 31 + N].rearrange("c (i j) -> c i j", j=31)
                    nc.vector.tensor_copy(l3[:, :, 1:31], c3[:, :, 0:30])
                    nc.vector.tensor_copy(r3[:, :, 0:30], c3[:, :, 1:31])

                # ---------- phase B: conv + softmax + spec + eigT ----------
                for (t, n0, row0, nrows, cnt) in (TILES if STAGE >= 3 else []):
                    fxp = psA.tile([124, 512], dt.float32, tag="a")
                    lgp = psB.tile([124, 512], dt.float32, tag="b")
                    first = True
                    for k in range(2):
                        for tap in range(9):
                            di, dj = tap // 3, tap % 3
                            base = 31 + (row0 + di - 1) * 31
                            lhsT = xq[k][dj][:, base : base + cnt]
                            nc.tensor.matmul(
                                fxp[:cnt, :],
                                lhsT,
                                wc_s[k][:, tap * 1024 : tap * 1024 + 512],
                                start=first,
                                stop=False,
                            )
                            nc.tensor.matmul(
                                lgp[:cnt, :],
                                lhsT,
                                wc_s[k][:, tap * 1024 + 512 : tap * 1024 + 1024],
                                start=first,
                                stop=False,
                            )
                            first = False
                    nc.tensor.matmul(
                        fxp[:cnt, :], ones_b[:1, :cnt], cb_s[:1, :512], start=False, stop=True
                    )
                    nc.tensor.matmul(
                        lgp[:cnt, :], ones_b[:1, :cnt], cb_s[:1, 512:], start=False, stop=True
                    )
                    fx_t = fxsb.tile([124, 512], dt.float16, tag="fx")
                    nc.scalar.copy(fx_t[:cnt, :], fxp[:cnt, :])
                    if STAGE < 4:
                        nc.scalar.copy(fx_t[:cnt, :], lgp[:cnt, :])
                        continue

                    # softmax over each head's 64 freqs (no max-sub needed; logits are O(1))
                    ex = expsb.tile([124, 512], dt.float32, tag="ex")
                    sm = smsb.tile([124, 8], dt.float32, tag="sm")
                    for h in range(8):
                        nc.scalar.activation(
                            ex[:cnt, h * 64 : (h + 1) * 64],
                            lgp[:cnt, h * 64 : (h + 1) * 64],
                            AF.Exp,
                            accum_out=sm[:cnt, h : h + 1],
                        )
                    rs = smsb.tile([124, 8], dt.float32, tag="rs")
                    nc.vector.reciprocal(rs[:cnt, :], sm[:cnt, :])
                    eg = eigsb.tile([124, 512], dt.float16, tag="eg")
                    for h in range(8):
                        hs = slice(h * 64, (h + 1) * 64)
                        nc.vector.tensor_mul(
                            ex[:cnt, hs], ex[:cnt, hs],
                            inv_s[:cnt, t * 64 : (t + 1) * 64],
                        )
                        nc.vector.tensor_scalar(
                            eg[:cnt, hs], ex[:cnt, hs], rs[:cnt, h : h + 1], None, ALU.mult
                        )

                    # spec accumulation (head pairs, block matmul)
                    if STAGE < 5:
                        continue
                    for p in range(4):
                        ps = slice(p * 128, (p + 1) * 128)
                        sp = psC.tile([128, 128], dt.float32, tag="c")
                        nc.tensor.matmul(
                            sp[:, :], eg[:cnt, ps], fx_t[:cnt, ps], start=True, stop=True
                        )
                        if t == 0:
                            nc.vector.tensor_copy(spec_acc[:, ps], sp[:, :])
                        else:
                            nc.vector.tensor_add(spec_acc[:, ps], spec_acc[:, ps], sp[:, :])

                    # transpose eig into eigT
                    for s in range(4):
                        ss = slice(s * 128, (s + 1) * 128)
                        tp = psC.tile([128, 128], dt.float16, tag="c")
                        nc.tensor.transpose(tp[:128, :cnt], eg[:cnt, ss], id_b[:cnt, :cnt])
                        nc.scalar.copy(eigT[s][:, n0 : n0 + cnt], tp[:, :cnt])

                # ---------- LayerNorm over (g,c) per head + mlp + F ----------
                if STAGE < 6:
                    continue
                # specT pairs with off-diagonal quadrants zeroed so full-width
                # base-0 ones-matmuls give per-(h, g) column sums (and the
                # partition broadcast of the stats for free).
                stp = [lnsb.tile([128, 128], dt.float32, tag=f"stp{p}", name=f"stp{p}") for p in range(4)]
                sq = lnsb.tile([128, 128], dt.float32, tag="sq")
                s1v = lnsb.tile([128, 16], dt.float32, tag="s1v")  # [0:8]=S1 [8:16]=S2
                for p in range(4):
                    ps = slice(p * 128, (p + 1) * 128)
                    tp = psB.tile([128, 128], dt.float32, tag="b")
                    nc.tensor.transpose(tp[:, :], spec_acc[:, ps], id_f[:, :128])
                    nc.gpsimd.memset(stp[p][:, :], 0.0)
                    for q in range(2):
                        qp = slice(q * 64, (q + 1) * 64)
                        nc.scalar.copy(stp[p][qp, qp], tp[qp, qp])
                    nc.scalar.square(sq[:, :], stp[p][:, :])
                    if KSUB < 1:
                        continue
                    s1p = psB.tile([128, 128], dt.float32, tag="b")
                    s2p = psA.tile([128, 128], dt.float32, tag="a")
                    nc.tensor.matmul(s1p[:, :], ones_m[:, :], stp[p][:, :], start=True, stop=True)
                    nc.tensor.matmul(s2p[:, :], ones_m[:, :], sq[:, :], start=True, stop=True)
                    for q in range(2):
                        h = 2 * p + q
                        qp = slice(q * 64, (q + 1) * 64)
                        nc.vector.reduce_sum(
                            s1v[:, h : h + 1], s1p[:, qp], axis=mybir.AxisListType.X
                        )
                        nc.vector.reduce_sum(
                            s1v[:, 8 + h : 9 + h], s2p[:, qp], axis=mybir.AxisListType.X
                        )
                # stats replicated across all 128 partitions
                if KSUB < 2:
                    continue
                mu = lnsb.tile([128, 8], dt.float32, tag="mu")
                nc.vector.tensor_scalar(mu[:, :], s1v[:, :8], 1.0 / 4096.0, None, ALU.mult)
                ex2 = lnsb.tile([128, 8], dt.float32, tag="ex2")
                nc.vector.tensor_scalar(ex2[:, :], s1v[:, 8:], 1.0 / 4096.0, None, ALU.mult)
                musq = lnsb.tile([128, 8], dt.float32, tag="musq")
                nc.vector.tensor_mul(musq[:, :], mu[:, :], mu[:, :])
                var = lnsb.tile([128, 8], dt.float32, tag="var")
                nc.vector.tensor_sub(var[:, :], ex2[:, :], musq[:, :])
                stdv = lnsb.tile([128, 8], dt.float32, tag="stdv")
                nc.scalar.activation(stdv[:, :], var[:, :], AF.Sqrt, bias=eps_t[:, :1])
                rstd = lnsb.tile([128, 8], dt.float32, tag="rstd")
                nc.vector.reciprocal(rstd[:, :], stdv[:, :])

                if KSUB < 4:
                    continue
                stn8 = lnsb.tile([DH, 8 * DH], dt.float16, tag="stn8")
                ost8 = lnsb.tile([DH, 8 * DH], dt.float16, tag="ost8")
                for p in range(4):
                    stn = lnsb.tile([128, 128], dt.float16, tag=f"stn{p}", name=f"stn{p}")
                    for q in range(2):
                        h = 2 * p + q
                        qp = slice(q * 64, (q + 1) * 64)
                        nc.vector.tensor_scalar(
                            stp[p][qp, qp], stp[p][qp, qp],
                            mu[qp, h : h + 1], rstd[qp, h : h + 1],
                            ALU.subtract, ALU.mult,
                        )
                        nc.vector.tensor_mul(stp[p][qp, qp], stp[p][qp, qp], gam_s[qp, :])
                        nc.vector.tensor_add(stn[qp, qp], stp[p][qp, qp], bet_s[qp, :])
                    # gather normalized quadrants at base partition 0
                    nc.scalar.copy(stn8[:, (2 * p) * 64 : (2 * p + 1) * 64], stn[:64, :64])
                    nc.sync.dma_start(
                        stn8[:, (2 * p + 1) * 64 : (2 * p + 2) * 64], stn[64:128, 64:128]
                    )
                # mlp per head: out_specT[h] = mlp_w^T-contraction (all base 0)
                for h in (range(8) if KSUB >= 5 else []):
                    op_ = psB.tile([DH, DH], dt.float32, tag="b")
                    nc.tensor.matmul(
                        op_[:, :], mlp_s[:64, :], stn8[:, h * 64 : (h + 1) * 64],
                        start=True, stop=True,
                    )
                    nc.scalar.copy(ost8[:, h * 64 : (h + 1) * 64], op_[:, :])
                # F[hg, co] per head (all base 0; odd heads shifted via DMA)
                for h in (range(8) if KSUB >= 6 else []):
                    fp = psA.tile([64, C], dt.float32, tag="a")
                    nc.tensor.matmul(
                        fp[:, :], ost8[:, h * 64 : (h + 1) * 64],
                        ow_s[:, h * C : (h + 1) * C], start=True, stop=True,
                    )
                    if h % 2 == 0:
                        nc.scalar.copy(F_sb[h // 2][:64, :], fp[:, :])
                    else:
                        fstg = lnsb.tile([64, C], dt.float16, tag="fstg")
                        nc.scalar.copy(fstg[:, :], fp[:, :])
                        nc.sync.dma_start(F_sb[h // 2][64:128, :], fstg[:, :])

                # ---------- phase C: out = eigT^T @ F + out_b ----------
                for (t, n0, row0, nrows, cnt) in (TILES if STAGE >= 7 else []):
                    op_ = psA.tile([124, C], dt.float32, tag="a")
                    for s in range(4):
                        nc.tensor.matmul(
                            op_[:cnt, :], eigT[s][:, n0 : n0 + cnt], F_sb[s][:, :],
                            start=(s == 0), stop=False,
                        )
                    nc.tensor.matmul(
                        op_[:cnt, :], ones_b[:1, :cnt], ob_s[:1, :], start=False, stop=True
                    )
                    ot = outsb.tile([124, C], dt.float16, tag="ot")
                    nc.scalar.copy(ot[:cnt, :], op_[:cnt, :])
                    nc.sync.dma_start(out_d[ds(iv * N + n0, cnt), :], ot[:cnt, :])
              if ctx_loop is not None:
                ctx_loop.__exit__(None, None, None)

    nc.compile()
    return nc


def _host_prep(conv_fx_w, conv_fx_b, conv_x_w, conv_x_b, gate_w, gate_b,
               temperature, ln_gamma, ln_beta, mlp_w, out_w, out_b, inver):
    f16 = np.float16

    temp = np.clip(np.asarray(temperature, np.float32).reshape(HEADS), 0.1, 5.0)
    # Wbig[cout, h*64+g] = gate_w[g, cout%64... block-diag per head] / temp_h
    gw = np.asarray(gate_w, np.float32)          # (FREQ, DH) = (g, dh)
    wbig = np.zeros((INNER, INNER), np.float32)
    for h in range(HEADS):
        wbig[h * DH : (h + 1) * DH, h * FREQ : (h + 1) * FREQ] = gw.T / temp[h]
    # fused logits conv weights + bias
    wx = np.asarray(conv_x_w, np.float32)        # (cout, cin, 3, 3)
    wlog = np.einsum("oidj,oF->djiF", wx, wbig)  # (3,3,256,512)
    logb = np.asarray(conv_x_b, np.float32) @ wbig
    logb = logb + np.repeat(np.asarray(gate_b, np.float32)[None, :], HEADS, 0).reshape(-1) / np.repeat(temp, FREQ)
    wfx = np.asarray(conv_fx_w, np.float32).transpose(2, 3, 1, 0)  # (3,3,256,512)
    # combined (tap-major within k-half): (2, 128, 9, 1024)
    wc = np.concatenate([wfx, wlog], axis=-1)    # (3,3,256,1024)
    wc = wc.reshape(9, 2, 128, 1024).transpose(1, 2, 0, 3).reshape(2, 128, 9 * 1024)
    cbias = np.concatenate([np.asarray(conv_fx_b, np.float32), logb])[None, :]

    gamT = np.asarray(ln_gamma, np.float32).T    # (c, g)
    betT = np.asarray(ln_beta, np.float32).T
    mlp_rep = np.vstack([np.asarray(mlp_w, np.float32)] * 2)       # (128, 64)
    ow = np.asarray(out_w, np.float32)           # (256, 512)
    owt = ow.reshape(C, HEADS, DH).transpose(2, 1, 0).reshape(DH, HEADS * C)

    pack = np.empty(WPACK_LEN, np.float16)
    pieces = [
        (OFF_WC, wc), (OFF_CB, cbias), (OFF_INV, np.asarray(inver, np.float32)),
        (OFF_MLP, mlp_rep), (OFF_GAM, np.vstack([gamT, gamT])),
        (OFF_BET, np.vstack([betT, betT])), (OFF_OW, owt),
        (OFF_OB, np.asarray(out_b, np.float32)[None, :]),
    ]
    for off, arr in pieces:
        flat = np.asarray(arr, np.float32).reshape(-1)
        pack[off : off + flat.size] = flat.astype(np.float16)
    return {"wpack": pack}


def kernel(x, conv_fx_w, conv_fx_b, conv_x_w, conv_x_b, gate_w, gate_b,
           temperature, ln_gamma, ln_beta, mlp_w, out_w, out_b, inver):
    global _BUILT
    import time as _time
    import sys as _sys
    _t0 = _time.time()
    import concourse.bass2jax  # noqa: F401  (primes the exec path)
    _t1 = _time.time()
    if _BUILT is None:
        _BUILT = _build_program(BPC // WAVES)
    nc = _BUILT
    _t2 = _time.time()

    weights = _host_prep(conv_fx_w, conv_fx_b, conv_x_w, conv_x_b, gate_w, gate_b,
                         temperature, ln_gamma, ln_beta, mlp_w, out_w, out_b, inver)
    x = np.asarray(x).reshape(NCORES * BPC * N, C).astype(np.float16)

    _t3 = _time.time()
    out = _run_pjrt(nc, x, weights)
    _t4 = _time.time()
    out = out.reshape(NCORES * BPC, N, C).astype(np.float32)
    print(f"[kernel] imports={_t1-_t0:.2f}s build={_t2-_t1:.2f}s prep={_t3-_t2:.2f}s run={_t4-_t3:.2f}s gather={_time.time()-_t4:.2f}s",
          file=_sys.stderr, flush=True)
    return out

_JITTED = None


def _run_pjrt(nc, x_global, weights):
    """Sharded bass_exec run, split into KWAVES async waves so wave N's
    upload overlaps wave N-1's execute/download.  x/out sharded over cores,
    weights replicated, donated output buffers created on-device."""
    global _JITTED
    import jax
    import jax.numpy as jnp
    from jax.experimental.shard_map import shard_map
    from jax.sharding import Mesh, NamedSharding, PartitionSpec as P
    import concourse.mybir as mybir
    from concourse import bass2jax

    bass2jax.install_neuronx_cc_hook()
    try:
        jax.config.update("jax_compilation_cache_dir", "/tmp/jax_comp_cache")
        jax.config.update("jax_persistent_cache_min_compile_time_secs", 0.0)
        jax.config.update("jax_persistent_cache_min_entry_size_bytes", -1)
    except Exception:
        pass

    bpc = BPC // WAVES
    pname = nc.partition_id_tensor.name if nc.partition_id_tensor else None
    in_names = []
    out_names = []
    out_shapes = []
    for alloc in nc.m.functions[0].allocations:
        if not isinstance(alloc, mybir.MemoryLocationSet):
            continue
        name = alloc.memorylocations[0].name
        if alloc.kind == "ExternalInput":
            if name != pname:
                in_names.append(name)
        elif alloc.kind == "ExternalOutput":
            out_shapes.append((tuple(alloc.tensor_shape), mybir.dt.np(alloc.dtype)))
            out_names.append(name)
    assert out_names == ["out"]
    import jax.core
    out_avals = [jax.core.ShapedArray(sh, dt) for sh, dt in out_shapes]
    all_in = list(in_names) + list(out_names)
    if pname is not None:
        all_in.append(pname)

    mesh = Mesh(np.asarray(jax.devices()[:NCORES]), ("core",))

    if _JITTED is None:
        def _body(*args):
            operands = list(args)
            if pname is not None:
                operands.append(bass2jax.partition_id_tensor())
            outs = bass2jax._bass_exec_p.bind(
                *operands,
                out_avals=tuple(out_avals),
                in_names=tuple(all_in),
                out_names=tuple(out_names),
                lowering_input_output_aliases=(),
                sim_require_finite=True,
                sim_require_nnan=True,
                nc=nc,
            )
            return tuple(outs)

        in_specs = tuple(
            P("core") if nm == "x" else P() for nm in in_names
        ) + (P("core"),)
        donate = (len(in_names),)
        sharded = jax.jit(
            shard_map(_body, mesh=mesh, in_specs=in_specs,
                      out_specs=(P("core"),), check_rep=False),
            donate_argnums=donate, keep_unused=True,
        )
        zfn = jax.jit(
            lambda: jnp.zeros((NCORES * bpc * N, C), jnp.float16),
            out_shardings=NamedSharding(mesh, P("core")),
        )
        _JITTED = (sharded, zfn)
    sharded, zfn = _JITTED

    rep = NamedSharding(mesh, P())
    shardspec = NamedSharding(mesh, P("core"))
    gat = jax.jit(lambda v: v.reshape(-1), out_shardings=rep)
    wdev = {"wpack": gat(jax.device_put(weights["wpack"].reshape(NCORES, -1), shardspec))}
    xw = x_global.reshape(WAVES, NCORES * bpc * N, C)
    pend = []
    for w in range(WAVES):
        args = [xw[w] if nm == "x" else wdev[nm] for nm in in_names]
        (oa,) = sharded(*args, zfn())
        pend.append(oa)
    outs = [np.asarray(oa) for oa in pend]
    return np.stack(outs, 0) if WAVES > 1 else outs[0]


# revision 25
# speedup vs baseline: 3.1917x; 1.2055x over previous
"""Calibrated Spectral Mixer on 8 TRN2 NeuronCores (Bass/Tile).

Data-parallel over batch: 32 samples -> 4 per core.  Per sample:
  1. x (N,256) is PE-transposed into a zero-padded channel-major image
     xpT (256, 103*33) so the two 3x3 convs become 9 tap-shifted matmuls.
  2. fx conv and the FUSED (conv_x @ blockdiag(gate_w/temp)) "logits conv"
     are computed straight from xpT in (n, cout) orientation.
  3. softmax(logits) * inver -> eig (n-major), PE-transposed into eigT.
  4. spec = fx^T-contraction with eig via head-pair block matmuls,
     accumulated in SBUF; LayerNorm over (g,c); mlp; then the output
     projection is fused host-side-style on device:
     F[hg,co] = out_specT @ out_wT per head, out = eigT^T @ F + out_b.
"""

import numpy as np

H, W = 101, 31
HEADS, DH, FREQ = 8, 64, 64
C = 256
INNER = HEADS * DH          # 512
N = H * W                   # 3131
NCORES = 8
BPC = 4                     # samples per core
EPS = 1e-5
NLCR = 31 + N + 31          # one pad image-row at each end, flat layout

# n-tiles: 4 image rows (124 positions) each, last tile 1 row (31)
TILES = [(t, 124 * t, 4 * t, 4, 124) for t in range(25)] + [(25, 3100, 100, 1, 31)]
# (idx, n0, row0, nrows, cnt)

OFF_WC = 0
OFF_CB = OFF_WC + 2 * 128 * 9216
OFF_INV = OFF_CB + 1024
OFF_MLP = OFF_INV + N * FREQ
OFF_GAM = OFF_MLP + 128 * DH
OFF_BET = OFF_GAM + 128 * FREQ
OFF_OW = OFF_BET + 128 * FREQ
OFF_OB = OFF_OW + DH * 8 * C
WPACK_LEN = OFF_OB + C
assert WPACK_LEN % 8 == 0

_BUILT = None
import os
WAVES = int(os.environ.get("KWAVES", "1"))
UNROLL_SAMPLES = os.environ.get("KUNROLL", "1") == "1"
STAGE = int(os.environ.get("KSTAGE", "7"))
KSUB = int(os.environ.get("KSUB", "9"))


def _build_program(bpc=BPC):
    import concourse.bacc as bacc
    import concourse.bass as bass
    import concourse.mybir as mybir
    from concourse.tile import TileContext
    from concourse.masks import make_identity

    dt = mybir.dt
    AF = mybir.ActivationFunctionType
    ALU = mybir.AluOpType
    ds = bass.ds

    nc = bacc.Bacc(None, target_bir_lowering=False)

    x_d = nc.declare_dram_parameter("x", (bpc * N, C), dt.float16, isOutput=False)
    wp_d = nc.declare_dram_parameter("wpack", (WPACK_LEN,), dt.float16, isOutput=False)
    out_d = nc.declare_dram_parameter("out", (bpc * N, C), dt.float16, isOutput=True)

    def wslice(off, ln):
        return wp_d[off : off + ln]

    with TileContext(nc) as tc:
        with (
            tc.tile_pool(name="consts", bufs=1) as consts,
            tc.tile_pool(name="pers", bufs=1) as pers,
            tc.tile_pool(name="xload", bufs=3) as xload,
            tc.tile_pool(name="fxsb", bufs=2) as fxsb,
            tc.tile_pool(name="expsb", bufs=2) as expsb,
            tc.tile_pool(name="eigsb", bufs=2) as eigsb,
            tc.tile_pool(name="smsb", bufs=2) as smsb,
            tc.tile_pool(name="outsb", bufs=3) as outsb,
            tc.tile_pool(name="lnsb", bufs=1) as lnsb,
            tc.tile_pool(name="psA", bufs=2, space="PSUM") as psA,
            tc.tile_pool(name="psB", bufs=2, space="PSUM") as psB,
            tc.tile_pool(name="psC", bufs=4, space="PSUM") as psC,
        ):
            # ---- constants ----
            wc_s = [consts.tile([128, 9 * 1024], dt.float16, tag=f"wc{k}", name=f"wc{k}") for k in range(2)]
            for k in range(2):
                nc.sync.dma_start(
                    wc_s[k][:],
                    wslice(OFF_WC + k * 128 * 9216, 128 * 9216).rearrange("(p f) -> p f", f=9216),
                )
            cb_s = consts.tile([1, 1024], dt.float16, tag="cb")
            nc.sync.dma_start(cb_s[:], wslice(OFF_CB, 1024).rearrange("(p f) -> p f", p=1))
            inv_h = consts.tile([124, 26 * FREQ], dt.float16, tag="invh")
            nc.sync.dma_start(
                inv_h[:, : 25 * FREQ].rearrange("p (t g) -> p t g", g=FREQ),
                wslice(OFF_INV, 25 * 124 * FREQ).rearrange("(t p g) -> p t g", p=124, g=FREQ),
            )
            nc.sync.dma_start(
                inv_h[:31, 25 * FREQ :],
                wslice(OFF_INV + 25 * 124 * FREQ, 31 * FREQ).rearrange("(p g) -> p g", g=FREQ),
            )
            inv_s = consts.tile([124, 26 * FREQ], dt.float32, tag="inv")
            nc.vector.tensor_copy(inv_s[:, :], inv_h[:, :])
            mlp_s = consts.tile([128, DH], dt.float16, tag="mlp")
            nc.sync.dma_start(mlp_s[:], wslice(OFF_MLP, 128 * DH).rearrange("(p f) -> p f", f=DH))
            gam_h = consts.tile([128, FREQ], dt.float16, tag="gamh")
            nc.sync.dma_start(gam_h[:], wslice(OFF_GAM, 128 * FREQ).rearrange("(p f) -> p f", f=FREQ))
            gam_s = consts.tile([128, FREQ], dt.float32, tag="gam")
            nc.vector.tensor_copy(gam_s[:, :], gam_h[:, :])
            bet_h = consts.tile([128, FREQ], dt.float16, tag="beth")
            nc.sync.dma_start(bet_h[:], wslice(OFF_BET, 128 * FREQ).rearrange("(p f) -> p f", f=FREQ))
            bet_s = consts.tile([128, FREQ], dt.float32, tag="bet")
            nc.vector.tensor_copy(bet_s[:, :], bet_h[:, :])
            ow_s = consts.tile([DH, 8 * C], dt.float16, tag="ow")
            nc.sync.dma_start(ow_s[:], wslice(OFF_OW, DH * 8 * C).rearrange("(p f) -> p f", f=8 * C))
            ob_s = consts.tile([1, C], dt.float16, tag="ob")
            nc.sync.dma_start(ob_s[:], wslice(OFF_OB, C).rearrange("(p f) -> p f", p=1))

            id_f = consts.tile([128, 128], dt.float32, tag="idf")
            make_identity(nc, id_f)
            id_b = consts.tile([128, 128], dt.float16, tag="idb")
            make_identity(nc, id_b)
            ones_b = consts.tile([1, 128], dt.float16, tag="onb")
            nc.gpsimd.memset(ones_b[:], 1.0)
            ones_cf = consts.tile([128, 1], dt.float32, tag="oncf")
            nc.gpsimd.memset(ones_cf[:], 1.0)
            ones_rf = consts.tile([1, 128], dt.float32, tag="onrf")
            nc.gpsimd.memset(ones_rf[:], 1.0)
            eps_t = consts.tile([128, 1], dt.float32, tag="eps")
            nc.gpsimd.memset(eps_t[:], EPS)
            ones_m = consts.tile([128, 128], dt.float32, tag="onm")
            nc.gpsimd.memset(ones_m[:], 1.0)

            # ---- persistent per-sample buffers ----
            # xq[k][d]: channel-major x, column-shifted by (d-1), one zero
            # image-row of padding at each end; tap (di,dj) of the conv is the
            # contiguous slice xq[k][dj][:, 31 + (row0+di-1)*31 : +cnt].
            xq = [[pers.tile([128, NLCR], dt.float16, tag=f"xq{k}{d}", name=f"xq{k}{d}")
                   for d in range(3)] for k in range(2)]
            for k in range(2):
                for d in range(3):
                    nc.gpsimd.memset(xq[k][d][:], 0.0)
            eigT = [pers.tile([128, N], dt.float16, tag=f"eigT{s}", name=f"eigT{s}") for s in range(4)]
            spec_acc = pers.tile([128, 4 * 128], dt.float32, tag="spacc")
            F_sb = [pers.tile([128, C], dt.float16, tag=f"F{p}", name=f"Fsb{p}") for p in range(4)]

            for iv in (list(range(bpc)) if UNROLL_SAMPLES else [None]):
              ctx_loop = tc.For_i(0, bpc, 1) if iv is None else None
              if ctx_loop is not None:
                iv = ctx_loop.__enter__()
              if True:
                # ---------- phase A: transpose x into channel-major + shifts ----------
                for (t, n0, row0, nrows, cnt) in (TILES if STAGE >= 2 else []):
                    xt = xload.tile([124, C], dt.float16, tag="xt")
                    nc.sync.dma_start(xt[:cnt, :], x_d[ds(iv * N + n0, cnt), :])
                    for k in range(2):
                        tp = psA.tile([128, 128], dt.float16, tag="a")
                        nc.tensor.transpose(
                            tp[:128, :cnt], xt[:cnt, k * 128 : (k + 1) * 128], id_b[:cnt, :cnt]
                        )
                        nc.scalar.copy(xq[k][1][:, 31 + n0 : 31 + n0 + cnt], tp[:, :cnt])
                for k in (range(2) if STAGE >= 2 else []):
                    c3 = xq[k][1][:, 31 : 31 + N].rearrange("c (i j) -> c i j", j=31)
                    l3 = xq[k][0][:, 31 : 31 + N].rearrange("c (i j) -> c i j", j=31)
                    r3 = xq[k][2][:, 31 : 31 + N].rearrange("c (i j) -> c i j", j=31)
                    nc.vector.tensor_copy(l3[:, :, 1:31], c3[:, :, 0:30])
                    nc.vector.tensor_copy(r3[:, :, 0:30], c3[:, :, 1:31])

                # ---------- phase B: conv + softmax + spec + eigT ----------
                for (t, n0, row0, nrows, cnt) in (TILES if STAGE >= 3 else []):
                    fxp = psA.tile([124, 512], dt.float32, tag="a")
                    lgp = psB.tile([124, 512], dt.float32, tag="b")
                    first = True
                    for k in range(2):
                        for tap in range(9):
                            di, dj = tap // 3, tap % 3
                            base = 31 + (row0 + di - 1) * 31
                            lhsT = xq[k][dj][:, base : base + cnt]
                            nc.tensor.matmul(
                                fxp[:cnt, :],
                                lhsT,
                                wc_s[k][:, tap * 1024 : tap * 1024 + 512],
                                start=first,
                                stop=False,
                            )
                            nc.tensor.matmul(
                                lgp[:cnt, :],
                                lhsT,
                                wc_s[k][:, tap * 1024 + 512 : tap * 1024 + 1024],
                                start=first,
                                stop=False,
                            )
                            first = False
                    nc.tensor.matmul(
                        fxp[:cnt, :], ones_b[:1, :cnt], cb_s[:1, :512], start=False, stop=True
                    )
                    nc.tensor.matmul(
                        lgp[:cnt, :], ones_b[:1, :cnt], cb_s[:1, 512:], start=False, stop=True
                    )
                    fx_t = fxsb.tile([124, 512], dt.float16, tag="fx")
                    nc.scalar.copy(fx_t[:cnt, :], fxp[:cnt, :])
                    if STAGE < 4:
                        nc.scalar.copy(fx_t[:cnt, :], lgp[:cnt, :])
                        continue

                    # softmax over each head's 64 freqs (no max-sub needed; logits are O(1))
                    ex = expsb.tile([124, 512], dt.float32, tag="ex")
                    sm = smsb.tile([124, 8], dt.float32, tag="sm")
                    for h in range(8):
                        nc.scalar.activation(
                            ex[:cnt, h * 64 : (h + 1) * 64],
                            lgp[:cnt, h * 64 : (h + 1) * 64],
                            AF.Exp,
                            accum_out=sm[:cnt, h : h + 1],
                        )
                    rs = smsb.tile([124, 8], dt.float32, tag="rs")
                    nc.vector.reciprocal(rs[:cnt, :], sm[:cnt, :])
                    eg = eigsb.tile([124, 512], dt.float16, tag="eg")
                    for h in range(8):
                        hs = slice(h * 64, (h + 1) * 64)
                        nc.vector.tensor_mul(
                            ex[:cnt, hs], ex[:cnt, hs],
                            inv_s[:cnt, t * 64 : (t + 1) * 64],
                        )
                        nc.vector.tensor_scalar(
                            eg[:cnt, hs], ex[:cnt, hs], rs[:cnt, h : h + 1], None, ALU.mult
                        )

                    # spec accumulation (head pairs, block matmul)
                    if STAGE < 5:
                        continue
                    for p in range(4):
                        ps = slice(p * 128, (p + 1) * 128)
                        sp = psC.tile([128, 128], dt.float32, tag="c")
                        nc.tensor.matmul(
                            sp[:, :], eg[:cnt, ps], fx_t[:cnt, ps], start=True, stop=True
                        )
                        if t == 0:
                            nc.vector.tensor_copy(spec_acc[:, ps], sp[:, :])
                        else:
                            nc.vector.tensor_add(spec_acc[:, ps], spec_acc[:, ps], sp[:, :])

                    # transpose eig into eigT
                    for s in range(4):
                        ss = slice(s * 128, (s + 1) * 128)
                        tp = psC.tile([128, 128], dt.float16, tag="c")
                        nc.tensor.transpose(tp[:128, :cnt], eg[:cnt, ss], id_b[:cnt, :cnt])
                        nc.scalar.copy(eigT[s][:, n0 : n0 + cnt], tp[:, :cnt])

                # ---------- LayerNorm over (g,c) per head + mlp + F ----------
                if STAGE < 6:
                    continue
                # specT pairs with off-diagonal quadrants zeroed so full-width
                # base-0 ones-matmuls give per-(h, g) column sums (and the
                # partition broadcast of the stats for free).
                stp = [lnsb.tile([128, 128], dt.float32, tag=f"stp{p}", name=f"stp{p}") for p in range(4)]
                sq = lnsb.tile([128, 128], dt.float32, tag="sq")
                s1v = lnsb.tile([128, 16], dt.float32, tag="s1v")  # [0:8]=S1 [8:16]=S2
                for p in range(4):
                    ps = slice(p * 128, (p + 1) * 128)
                    tp = psB.tile([128, 128], dt.float32, tag="b")
                    nc.tensor.transpose(tp[:, :], spec_acc[:, ps], id_f[:, :128])
                    nc.gpsimd.memset(stp[p][:, :], 0.0)
                    for q in range(2):
                        qp = slice(q * 64, (q + 1) * 64)
                        nc.scalar.copy(stp[p][qp, qp], tp[qp, qp])
                    nc.scalar.square(sq[:, :], stp[p][:, :])
                    if KSUB < 1:
                        continue
                    s1p = psB.tile([128, 128], dt.float32, tag="b")
                    s2p = psA.tile([128, 128], dt.float32, tag="a")
                    nc.tensor.matmul(s1p[:, :], ones_m[:, :], stp[p][:, :], start=True, stop=True)
                    nc.tensor.matmul(s2p[:, :], ones_m[:, :], sq[:, :], start=True, stop=True)
                    for q in range(2):
                        h = 2 * p + q
                        qp = slice(q * 64, (q + 1) * 64)
                        nc.vector.reduce_sum(
                            s1v[:, h : h + 1], s1p[:, qp], axis=mybir.AxisListType.X
                        )
                        nc.vector.reduce_sum(
                            s1v[:, 8 + h : 9 + h], s2p[:, qp], axis=mybir.AxisListType.X
                        )
                # stats replicated across all 128 partitions
                if KSUB < 2:
                    continue
                mu = lnsb.tile([128, 8], dt.float32, tag="mu")
                nc.vector.tensor_scalar(mu[:, :], s1v[:, :8], 1.0 / 4096.0, None, ALU.mult)
                ex2 = lnsb.tile([128, 8], dt.float32, tag="ex2")
                nc.vector.tensor_scalar(ex2[:, :], s1v[:, 8:], 1.0 / 4096.0, None, ALU.mult)
                musq = lnsb.tile([128, 8], dt.float32, tag="musq")
                nc.vector.tensor_mul(musq[:, :], mu[:, :], mu[:, :])
                var = lnsb.tile([128, 8], dt.float32, tag="var")
                nc.vector.tensor_sub(var[:, :], ex2[:, :], musq[:, :])
                stdv = lnsb.tile([128, 8], dt.float32, tag="stdv")
                nc.scalar.activation(stdv[:, :], var[:, :], AF.Sqrt, bias=eps_t[:, :1])
                rstd = lnsb.tile([128, 8], dt.float32, tag="rstd")
                nc.vector.reciprocal(rstd[:, :], stdv[:, :])

                if KSUB < 4:
                    continue
                stn8 = lnsb.tile([DH, 8 * DH], dt.float16, tag="stn8")
                ost8 = lnsb.tile([DH, 8 * DH], dt.float16, tag="ost8")
                for p in range(4):
                    stn = lnsb.tile([128, 128], dt.float16, tag=f"stn{p}", name=f"stn{p}")
                    for q in range(2):
                        h = 2 * p + q
                        qp = slice(q * 64, (q + 1) * 64)
                        nc.vector.tensor_scalar(
                            stp[p][qp, qp], stp[p][qp, qp],
                            mu[qp, h : h + 1], rstd[qp, h : h + 1],
                            ALU.subtract, ALU.mult,
                        )
                        nc.vector.tensor_mul(stp[p][qp, qp], stp[p][qp, qp], gam_s[qp, :])
                        nc.vector.tensor_add(stn[qp, qp], stp[p][qp, qp], bet_s[qp, :])
                    # gather normalized quadrants at base partition 0
                    nc.scalar.copy(stn8[:, (2 * p) * 64 : (2 * p + 1) * 64], stn[:64, :64])
                    nc.sync.dma_start(
                        stn8[:, (2 * p + 1) * 64 : (2 * p + 2) * 64], stn[64:128, 64:128]
                    )
                # mlp per head: out_specT[h] = mlp_w^T-contraction (all base 0)
                for h in (range(8) if KSUB >= 5 else []):
                    op_ = psB.tile([DH, DH], dt.float32, tag="b")
                    nc.tensor.matmul(
                        op_[:, :], mlp_s[:64, :], stn8[:, h * 64 : (h + 1) * 64],
                        start=True, stop=True,
                    )
                    nc.scalar.copy(ost8[:, h * 64 : (h + 1) * 64], op_[:, :])
                # F[hg, co] per head (all base 0; odd heads shifted via DMA)
                for h in (range(8) if KSUB >= 6 else []):
                    fp = psA.tile([64, C], dt.float32, tag="a")
                    nc.tensor.matmul(
                        fp[:, :], ost8[:, h * 64 : (h + 1) * 64],
                        ow_s[:, h * C : (h + 1) * C], start=True, stop=True,
                    )
                    if h % 2 == 0:
                        nc.scalar.copy(F_sb[h // 2][:64, :], fp[:, :])
                    else:
                        fstg = lnsb.tile([64, C], dt.float16, tag="fstg")
                        nc.scalar.copy(fstg[:, :], fp[:, :])
                        nc.sync.dma_start(F_sb[h // 2][64:128, :], fstg[:, :])

                # ---------- phase C: out = eigT^T @ F + out_b ----------
                for (t, n0, row0, nrows, cnt) in (TILES if STAGE >= 7 else []):
                    op_ = psA.tile([124, C], dt.float32, tag="a")
                    for s in range(4):
                        nc.tensor.matmul(
                            op_[:cnt, :], eigT[s][:, n0 : n0 + cnt], F_sb[s][:, :],
                            start=(s == 0), stop=False,
                        )
                    nc.tensor.matmul(
                        op_[:cnt, :], ones_b[:1, :cnt], ob_s[:1, :], start=False, stop=True
                    )
                    ot = outsb.tile([124, C], dt.float16, tag="ot")
                    nc.scalar.copy(ot[:cnt, :], op_[:cnt, :])
                    nc.sync.dma_start(out_d[ds(iv * N + n0, cnt), :], ot[:cnt, :])
              if ctx_loop is not None:
                ctx_loop.__exit__(None, None, None)

    nc.compile()
    return nc


def _host_prep(conv_fx_w, conv_fx_b, conv_x_w, conv_x_b, gate_w, gate_b,
               temperature, ln_gamma, ln_beta, mlp_w, out_w, out_b, inver):
    f16 = np.float16

    temp = np.clip(np.asarray(temperature, np.float32).reshape(HEADS), 0.1, 5.0)
    # Wbig[cout, h*64+g] = gate_w[g, cout%64... block-diag per head] / temp_h
    gw = np.asarray(gate_w, np.float32)          # (FREQ, DH) = (g, dh)
    wbig = np.zeros((INNER, INNER), np.float32)
    for h in range(HEADS):
        wbig[h * DH : (h + 1) * DH, h * FREQ : (h + 1) * FREQ] = gw.T / temp[h]
    # fused logits conv weights + bias
    wx = np.asarray(conv_x_w, np.float32)        # (cout, cin, 3, 3)
    wlog = np.einsum("oidj,oF->djiF", wx, wbig)  # (3,3,256,512)
    logb = np.asarray(conv_x_b, np.float32) @ wbig
    logb = logb + np.repeat(np.asarray(gate_b, np.float32)[None, :], HEADS, 0).reshape(-1) / np.repeat(temp, FREQ)
    wfx = np.asarray(conv_fx_w, np.float32).transpose(2, 3, 1, 0)  # (3,3,256,512)
    # combined (tap-major within k-half): (2, 128, 9, 1024)
    wc = np.concatenate([wfx, wlog], axis=-1)    # (3,3,256,1024)
    wc = wc.reshape(9, 2, 128, 1024).transpose(1, 2, 0, 3).reshape(2, 128, 9 * 1024)
    cbias = np.concatenate([np.asarray(conv_fx_b, np.float32), logb])[None, :]

    gamT = np.asarray(ln_gamma, np.float32).T    # (c, g)
    betT = np.asarray(ln_beta, np.float32).T
    mlp_rep = np.vstack([np.asarray(mlp_w, np.float32)] * 2)       # (128, 64)
    ow = np.asarray(out_w, np.float32)           # (256, 512)
    owt = ow.reshape(C, HEADS, DH).transpose(2, 1, 0).reshape(DH, HEADS * C)

    pack = np.empty(WPACK_LEN, np.float16)
    pieces = [
        (OFF_WC, wc), (OFF_CB, cbias), (OFF_INV, np.asarray(inver, np.float32)),
        (OFF_MLP, mlp_rep), (OFF_GAM, np.vstack([gamT, gamT])),
        (OFF_BET, np.vstack([betT, betT])), (OFF_OW, owt),
        (OFF_OB, np.asarray(out_b, np.float32)[None, :]),
    ]
    for off, arr in pieces:
        flat = np.asarray(arr, np.float32).reshape(-1)
        pack[off : off + flat.size] = flat.astype(np.float16)
    return {"wpack": pack}


def kernel(x, conv_fx_w, conv_fx_b, conv_x_w, conv_x_b, gate_w, gate_b,
           temperature, ln_gamma, ln_beta, mlp_w, out_w, out_b, inver):
    global _BUILT
    import time as _time
    import sys as _sys
    _t0 = _time.time()
    import concourse.bass2jax  # noqa: F401  (primes the exec path)
    _t1 = _time.time()
    if _BUILT is None:
        _BUILT = _get_program(BPC // WAVES)
    nc = _BUILT
    _t2 = _time.time()

    weights = _host_prep(conv_fx_w, conv_fx_b, conv_x_w, conv_x_b, gate_w, gate_b,
                         temperature, ln_gamma, ln_beta, mlp_w, out_w, out_b, inver)
    x = np.asarray(x).reshape(NCORES * BPC * N, C).astype(np.float16)

    _t3 = _time.time()
    out = _run_pjrt(nc, x, weights)
    _t4 = _time.time()
    out = out.reshape(NCORES * BPC, N, C).astype(np.float32)
    print(f"[kernel] imports={_t1-_t0:.2f}s build={_t2-_t1:.2f}s prep={_t3-_t2:.2f}s run={_t4-_t3:.2f}s gather={_time.time()-_t4:.2f}s",
          file=_sys.stderr, flush=True)
    return out

_PROG_CACHE = "/tmp/bass_spectral_mixer_v1.pkl"


class _ProgStub:
    """Duck-typed stand-in for the Bass object: carries exactly what the
    bass_exec jit lowering reads (BIR bytes, arch, flags, I/O metadata)."""

    class _M:
        def __init__(self, arch):
            self.arch = arch

    def __init__(self, d):
        self._json = d["bir"]
        self.m = _ProgStub._M(d["arch"])
        self.has_collectives = d["has_collectives"]
        self.target_bir_lowering = False
        self.dbg_addr = None
        self.dbg_callbacks = []
        self.io_meta = d["io_meta"]

    def to_json_bytes(self):
        return self._json


def _extract_io_meta(nc):
    import concourse.mybir as mybir

    pname = nc.partition_id_tensor.name if nc.partition_id_tensor else None
    in_names, out_names, out_shapes = [], [], []
    for alloc in nc.m.functions[0].allocations:
        if not isinstance(alloc, mybir.MemoryLocationSet):
            continue
        name = alloc.memorylocations[0].name
        if alloc.kind == "ExternalInput":
            if name != pname:
                in_names.append(name)
        elif alloc.kind == "ExternalOutput":
            out_shapes.append((tuple(alloc.tensor_shape), np.dtype(mybir.dt.np(alloc.dtype)).name))
            out_names.append(name)
    return {"pname": pname, "in_names": in_names, "out_names": out_names,
            "out_shapes": out_shapes}


def _get_program(bpc):
    import pickle

    try:
        with open(_PROG_CACHE, "rb") as f:
            d = pickle.load(f)
        if d.get("bpc") == bpc:
            return _ProgStub(d)
    except Exception:
        pass
    nc = _build_program(bpc)
    d = {
        "bir": nc.to_json_bytes(),
        "arch": nc.m.arch,
        "has_collectives": nc.has_collectives,
        "io_meta": _extract_io_meta(nc),
        "bpc": bpc,
    }
    try:
        import pickle
        with open(_PROG_CACHE + ".tmp", "wb") as f:
            pickle.dump(d, f)
        os.replace(_PROG_CACHE + ".tmp", _PROG_CACHE)
    except Exception:
        pass
    return _ProgStub(d)


_JITTED = None


def _run_pjrt(nc, x_global, weights):
    """Sharded bass_exec run, split into KWAVES async waves so wave N's
    upload overlaps wave N-1's execute/download.  x/out sharded over cores,
    weights replicated, donated output buffers created on-device."""
    global _JITTED
    import jax
    import jax.numpy as jnp
    from jax.experimental.shard_map import shard_map
    from jax.sharding import Mesh, NamedSharding, PartitionSpec as P
    from concourse import bass2jax

    bass2jax.install_neuronx_cc_hook()
    try:
        jax.config.update("jax_compilation_cache_dir", "/tmp/jax_comp_cache")
        jax.config.update("jax_persistent_cache_min_compile_time_secs", 0.0)
        jax.config.update("jax_persistent_cache_min_entry_size_bytes", -1)
    except Exception:
        pass

    bpc = BPC // WAVES
    meta = nc.io_meta
    pname = meta["pname"]
    in_names = meta["in_names"]
    out_names = meta["out_names"]
    assert out_names == ["out"]
    import jax.core
    out_avals = [jax.core.ShapedArray(sh, np.dtype(dtn)) for sh, dtn in meta["out_shapes"]]
    all_in = list(in_names) + list(out_names)
    if pname is not None:
        all_in.append(pname)

    mesh = Mesh(np.asarray(jax.devices()[:NCORES]), ("core",))

    if _JITTED is None:
        def _body(*args):
            operands = list(args)
            if pname is not None:
                operands.append(bass2jax.partition_id_tensor())
            outs = bass2jax._bass_exec_p.bind(
                *operands,
                out_avals=tuple(out_avals),
                in_names=tuple(all_in),
                out_names=tuple(out_names),
                lowering_input_output_aliases=(),
                sim_require_finite=True,
                sim_require_nnan=True,
                nc=nc,
            )
            return tuple(outs)

        in_specs = tuple(
            P("core") if nm == "x" else P() for nm in in_names
        ) + (P("core"),)
        donate = (len(in_names),)
        sharded = jax.jit(
            shard_map(_body, mesh=mesh, in_specs=in_specs,
                      out_specs=(P("core"),), check_rep=False),
            donate_argnums=donate, keep_unused=True,
        )
        zfn = jax.jit(
            lambda: jnp.zeros((NCORES * bpc * N, C), jnp.float16),
            out_shardings=NamedSharding(mesh, P("core")),
        )
        _JITTED = (sharded, zfn)
    sharded, zfn = _JITTED

    rep = NamedSharding(mesh, P())
    shardspec = NamedSharding(mesh, P("core"))
    gat = jax.jit(lambda v: v.reshape(-1), out_shardings=rep)
    wdev = {"wpack": gat(jax.device_put(weights["wpack"].reshape(NCORES, -1), shardspec))}
    xw = x_global.reshape(WAVES, NCORES * bpc * N, C)
    pend = []
    for w in range(WAVES):
        args = [xw[w] if nm == "x" else wdev[nm] for nm in in_names]
        (oa,) = sharded(*args, zfn())
        pend.append(oa)
    outs = [np.asarray(oa) for oa in pend]
    return np.stack(outs, 0) if WAVES > 1 else outs[0]
